# revision 20
# baseline (speedup 1.0000x reference)
"""Trainium2 Bass kernel for an 8-expert top-2 MoE layer with shared expert.

Sharding: expert-parallel. Each of the 8 cores owns one expert's FFN weights
plus a 1/8 slice (intermediate dim) of the shared expert. hidden_states and
the router are replicated; each core computes the router for all tokens in
fp32 (top-2 selection is precision-critical), the dense-masked FFN for its
own expert in bf16 (fp32 accumulation), and its shared-expert slice. Partial
outputs are summed with an on-device ReduceScatter; the host concatenates
the 8 token-slices. aux losses are computed redundantly on every core.

Self-contained: shapes hardcoded for B=2, S=2048, H=1024, I=2048, E=8.
"""

import sys

sys.path.insert(0, "/opt/trn_rl_repo")

from contextlib import ExitStack

import numpy as np

import concourse.bacc as bacc
import concourse.mybir as mybir
from concourse import masks, tile
from concourse.bass_types import AP
from concourse.bass_utils import run_bass_kernel_spmd

F32 = mybir.dt.float32
BF16 = mybir.dt.bfloat16
AF = mybir.ActivationFunctionType
OP = mybir.AluOpType

N_CORES = 8
H = 1024
I_DIM = 2048
E = 8
ISL = I_DIM // N_CORES  # shared-expert intermediate slice per core
KH = H // 128           # 8 k-tiles over H
KI = I_DIM // 128       # 16 k-tiles over I
EPS = 1e-6


def _bc_last(ap: AP, n: int) -> AP:
    """Broadcast an AP along a new innermost dim of size n (stride 0)."""
    return AP(ap.tensor, ap.offset, [list(x) for x in ap.ap] + [[0, n]])


def _bc_mid(ap: AP, n: int) -> AP:
    """[128, F] -> [128, n, F] broadcast on the middle dim (stride 0)."""
    a = [list(x) for x in ap.ap]
    return AP(ap.tensor, ap.offset, [a[0], [0, n], *a[1:]])


def build_program(T: int = 4096, tc_tokens: int = 1024, native_silu: bool = True):
    assert T % 1024 == 0 or T in (512,), T
    TC = min(tc_tokens, T)
    NT = T // 128          # token tiles
    NCH = T // TC          # ffn token chunks
    NTS = TC // 128        # token tiles per chunk
    TSL = T // N_CORES     # output slice per core after ReduceScatter

    nc = bacc.Bacc(
        "TRN2",
        target_bir_lowering=False,
        debug=False,
        enable_asserts=True,
        num_devices=N_CORES,
    )

    x_ext = nc.dram_tensor("x", [T, H], F32, kind="ExternalInput").ap()
    rwT_ext = nc.dram_tensor("rwT", [H, E], F32, kind="ExternalInput").ap()
    wg_ext = nc.dram_tensor("wg", [H, I_DIM], F32, kind="ExternalInput").ap()
    wu_ext = nc.dram_tensor("wu", [H, I_DIM], F32, kind="ExternalInput").ap()
    wd_ext = nc.dram_tensor("wd", [I_DIM, H], F32, kind="ExternalInput").ap()
    swg_ext = nc.dram_tensor("swg", [H, ISL], F32, kind="ExternalInput").ap()
    swu_ext = nc.dram_tensor("swu", [H, ISL], F32, kind="ExternalInput").ap()
    swd_ext = nc.dram_tensor("swd", [ISL, H], F32, kind="ExternalInput").ap()
    sel_ext = nc.dram_tensor("sel", [128, E], F32, kind="ExternalInput").ap()
    sgate_ext = nc.dram_tensor("sgate", [128, 1], F32, kind="ExternalInput").ap()

    out_ext = nc.dram_tensor("out_rs", [TSL, H], F32, kind="ExternalOutput").ap()
    aux_ext = nc.dram_tensor("aux", [1, 1], F32, kind="ExternalOutput").ap()

    with tile.TileContext(nc) as tc, ExitStack() as top:
        dram = top.enter_context(tc.tile_pool(name="dram", bufs=1, space="DRAM"))
        RSC = TC // N_CORES                   # rows per core per chunk
        out_accs = [dram.tile([TC, H], F32, name=f"acc{q}", tag=f"acc{q}") for q in range(NCH)]
        rs_outs = [dram.tile([RSC, H], F32, name=f"rsq{q}", tag=f"rsq{q}") for q in range(NCH)]
        wgc = dram.tile([KI, 128, KH * 128], BF16, name="wgc", tag="wgc")
        wuc = dram.tile([KI, 128, KH * 128], BF16, name="wuc", tag="wuc")

        const_pool = top.enter_context(tc.tile_pool(name="const", bufs=1))
        ident = const_pool.tile([128, 128], F32)
        masks.make_identity(nc, ident[:])
        ones = const_pool.tile([128, 1], F32)
        nc.vector.memset(ones[:], 1.0)
        rwT_sb = const_pool.tile([128, KH, E], F32)
        nc.sync.dma_start(out=rwT_sb[:], in_=rwT_ext.rearrange("(k p) e -> p k e", p=128))
        sel_sb = const_pool.tile([128, E], F32)
        nc.sync.dma_start(out=sel_sb[:], in_=sel_ext)
        sig_sb = const_pool.tile([128, 1], F32)
        sg_in = const_pool.tile([128, 1], F32)
        nc.sync.dma_start(out=sg_in[:], in_=sgate_ext)
        nc.scalar.activation(sig_sb[:], sg_in[:], AF.Sigmoid)

        # Resident activations / weights
        big_pool = top.enter_context(tc.tile_pool(name="resident", bufs=1))
        xT_bf = big_pool.tile([128, KH, T], BF16)          # x^T, bf16
        wd_sb = big_pool.tile([128, KI, H], BF16)          # wd tiles [i_k][i_p, h]
        swg_sb = big_pool.tile([128, KH, ISL], BF16)
        swu_sb = big_pool.tile([128, KH, ISL], BF16)
        swd_sb = big_pool.tile([128, ISL // 128, H], BF16)
        nc.gpsimd.dma_start(out=wd_sb[:], in_=wd_ext.rearrange("(k p) h -> p k h", p=128))
        nc.gpsimd.dma_start(out=swg_sb[:], in_=swg_ext.rearrange("(k p) i -> p k i", p=128))
        nc.gpsimd.dma_start(out=swu_sb[:], in_=swu_ext.rearrange("(k p) i -> p k i", p=128))
        nc.gpsimd.dma_start(out=swd_sb[:], in_=swd_ext.rearrange("(k p) h -> p k h", p=128))

        # Pre-convert wg/wu to bf16 cache in DRAM (overlaps phase A)
        with tc.tile_pool(name="wconv", bufs=4) as wconv:
            for i in range(KI):
                cg = wconv.tile([128, KH, 128], BF16, tag="cg")
                cu = wconv.tile([128, KH, 128], BF16, tag="cu")
                nc.gpsimd.dma_start(
                    out=cg[:],
                    in_=AP(wg_ext.tensor, i * 128, [[I_DIM, 128], [128 * I_DIM, KH], [1, 128]]),
                )
                nc.gpsimd.dma_start(
                    out=cu[:],
                    in_=AP(wu_ext.tensor, i * 128, [[I_DIM, 128], [128 * I_DIM, KH], [1, 128]]),
                )
                nc.scalar.dma_start(out=wgc[i], in_=cg[:])
                nc.scalar.dma_start(out=wuc[i], in_=cu[:])

        # Router tensors (fp32), persistent until aux is finalized
        rt_pool = top.enter_context(tc.tile_pool(name="router", bufs=1))
        logits_all = rt_pool.tile([128, NT, E], F32)
        probs = rt_pool.tile([128, NT, E], F32)
        mask2 = rt_pool.tile([128, NT, E], F32)
        pl = rt_pool.tile([128, NT, E], F32)
        zsq = rt_pool.tile([128, NT], F32)
        cwc_all = rt_pool.tile([128, NT], F32)

        # ---------------- Phase A: x load + transpose + router logits ----
        with (
            tc.tile_pool(name="xin", bufs=3) as xin_pool,
            tc.tile_pool(name="xtT", bufs=2) as xtT_pool,
            tc.tile_pool(name="trps", bufs=2, space="PSUM") as tr_pool,
            tc.tile_pool(name="lgps", bufs=2, space="PSUM") as lg_pool,
        ):
            for j in range(NT):
                xt = xin_pool.tile([128, H], F32)
                nc.sync.dma_start(out=xt[:], in_=x_ext[j * 128:(j + 1) * 128, :])
                tr_ps = tr_pool.tile([128, H], F32)
                for k in range(KH):
                    nc.tensor.transpose(
                        tr_ps[:, k * 128:(k + 1) * 128],
                        xt[:, k * 128:(k + 1) * 128],
                        ident[:],
                    )
                xtT = xtT_pool.tile([128, H], F32)
                nc.scalar.activation(xtT[:], tr_ps[:], AF.Copy)
                nc.vector.tensor_copy(
                    xT_bf[:, :, j * 128:(j + 1) * 128],
                    xtT[:].rearrange("p (k t) -> p k t", k=KH),
                )
                lg_ps = lg_pool.tile([128, E], F32)
                for k in range(KH):
                    nc.tensor.matmul(
                        lg_ps[:],
                        xtT[:, k * 128:(k + 1) * 128],
                        rwT_sb[:, k, :],
                        start=(k == 0),
                        stop=(k == KH - 1),
                    )
                nc.vector.tensor_scalar(
                    out=logits_all[:, j, :], in0=lg_ps[:],
                    scalar1=50.0, scalar2=-50.0, op0=OP.min, op1=OP.max,
                )

        # ---------------- Router math (batched over all tokens) ----------
        with tc.tile_pool(name="rtmp", bufs=1) as rt:
            mx = rt.tile([128, NT], F32)
            nc.vector.tensor_reduce(mx[:], logits_all[:], axis=mybir.AxisListType.X, op=OP.max)
            shifted = rt.tile([128, NT, E], F32)
            nc.vector.tensor_tensor(shifted[:], logits_all[:], _bc_last(mx[:], E), OP.subtract)
            exps = rt.tile([128, NT, E], F32)
            nc.scalar.activation(exps[:], shifted[:], AF.Exp)
            sums = rt.tile([128, NT], F32)
            nc.vector.tensor_reduce(sums[:], exps[:], axis=mybir.AxisListType.X, op=OP.add)
            rcp = rt.tile([128, NT], F32)
            nc.vector.reciprocal(rcp[:], sums[:])
            nc.vector.tensor_tensor(probs[:], exps[:], _bc_last(rcp[:], E), OP.mult)
            # z-loss: lse = ln(sum) + max, squared
            lns = rt.tile([128, NT], F32)
            nc.scalar.activation(lns[:], sums[:], AF.Ln)
            lse = rt.tile([128, NT], F32)
            nc.vector.tensor_tensor(lse[:], lns[:], mx[:], OP.add)
            nc.vector.tensor_tensor(zsq[:], lse[:], lse[:], OP.mult)
            # top-2 selection on LOGITS (same order as probs; avoids any
            # dependence of the selection on exp-LUT rounding)
            m1 = rt.tile([128, NT], F32)
            nc.vector.tensor_reduce(m1[:], logits_all[:], axis=mybir.AxisListType.X, op=OP.max)
            eq1 = rt.tile([128, NT, E], F32)
            nc.vector.tensor_tensor(eq1[:], logits_all[:], _bc_last(m1[:], E), OP.is_ge)
            pm = rt.tile([128, NT, E], F32)
            nc.vector.tensor_scalar_mul(pm[:], eq1[:], 1.0e9)
            nc.vector.tensor_tensor(pm[:], logits_all[:], pm[:], OP.subtract)
            m2 = rt.tile([128, NT], F32)
            nc.vector.tensor_reduce(m2[:], pm[:], axis=mybir.AxisListType.X, op=OP.max)
            nc.vector.tensor_tensor(mask2[:], logits_all[:], _bc_last(m2[:], E), OP.is_ge)
            # cw = probs * mask2 / (sum of selected probs + EPS)
            cw = rt.tile([128, NT, E], F32)
            nc.vector.tensor_tensor(cw[:], probs[:], mask2[:], OP.mult)
            den = rt.tile([128, NT], F32)
            nc.vector.tensor_reduce(den[:], cw[:], axis=mybir.AxisListType.X, op=OP.add)
            nc.vector.tensor_scalar_add(den[:], den[:], EPS)
            rd = rt.tile([128, NT], F32)
            nc.vector.reciprocal(rd[:], den[:])
            nc.vector.tensor_tensor(cw[:], cw[:], _bc_last(rd[:], E), OP.mult)
            cwsel = rt.tile([128, NT, E], F32)
            nc.vector.tensor_tensor(cwsel[:], cw[:], _bc_mid(sel_sb[:], NT), OP.mult)
            nc.vector.tensor_reduce(cwc_all[:], cwsel[:], axis=mybir.AxisListType.X, op=OP.add)
            # entropy terms
            psafe = rt.tile([128, NT, E], F32)
            nc.vector.tensor_scalar_max(psafe[:], probs[:], EPS)
            lp = rt.tile([128, NT, E], F32)
            nc.scalar.activation(lp[:], psafe[:], AF.Ln)
            nc.vector.tensor_tensor(pl[:], psafe[:], lp[:], OP.mult)

        # ---------------- Phase B: routed expert FFN (dense, masked) -----
        # ---------------- Phase C: shared expert slice --------------------
        with (
            tc.tile_pool(name="wstream", bufs=6) as wstream,
            tc.tile_pool(name="gact", bufs=2) as gact_pool,
            tc.tile_pool(name="hT", bufs=1) as hT_pool,
            tc.tile_pool(name="osb", bufs=3) as osb_pool,
            tc.tile_pool(name="gps", bufs=1, space="PSUM") as g_pool,
            tc.tile_pool(name="ups", bufs=1, space="PSUM") as u_pool,
            tc.tile_pool(name="ops", bufs=2, space="PSUM") as o_pool,
        ):
            hT = hT_pool.tile([128, KI, TC], BF16, tag="hT")
            hsT = hT_pool.tile([128, ISL // 128, TC], BF16, tag="hsT")

            def _silu_mul(dst, g_ps, u_ps):
                ga = gact_pool.tile([128, TC], BF16, tag="ga")
                if native_silu:
                    nc.scalar.activation(ga[:], g_ps[:], AF.Silu)
                else:
                    sgm = gact_pool.tile([128, TC], F32, tag="sgm")
                    nc.scalar.activation(sgm[:], g_ps[:], AF.Sigmoid)
                    nc.vector.tensor_tensor(ga[:], sgm[:], g_ps[:], OP.mult)
                nc.vector.tensor_tensor(dst, ga[:], u_ps[:], OP.mult)
            for ch in range(NCH):
                t0 = ch * TC
                # gate/up for the routed expert
                for i in range(KI):
                    wg_sb = wstream.tile([128, KH, 128], BF16, tag="wg")
                    wu_sb = wstream.tile([128, KH, 128], BF16, tag="wu")
                    nc.sync.dma_start(out=wg_sb[:].rearrange("p a b -> p (a b)"), in_=wgc[i])
                    nc.sync.dma_start(out=wu_sb[:].rearrange("p a b -> p (a b)"), in_=wuc[i])
                    g_ps = g_pool.tile([128, TC], F32, tag="g")
                    u_ps = u_pool.tile([128, TC], F32, tag="u")
                    for half in range(TC // 512):
                        hs = slice(half * 512, (half + 1) * 512)
                        for k in range(KH):
                            nc.tensor.matmul(
                                g_ps[:, hs], wg_sb[:, k, :],
                                xT_bf[:, k, t0 + half * 512: t0 + (half + 1) * 512],
                                start=(k == 0), stop=(k == KH - 1),
                            )
                        for k in range(KH):
                            nc.tensor.matmul(
                                u_ps[:, hs], wu_sb[:, k, :],
                                xT_bf[:, k, t0 + half * 512: t0 + (half + 1) * 512],
                                start=(k == 0), stop=(k == KH - 1),
                            )
                    _silu_mul(hT[:, i, :], g_ps, u_ps)
                # gate/up for the shared expert slice
                for i2 in range(ISL // 128):
                    g_ps = g_pool.tile([128, TC], F32, tag="g")
                    u_ps = u_pool.tile([128, TC], F32, tag="u")
                    for half in range(TC // 512):
                        hs = slice(half * 512, (half + 1) * 512)
                        for k in range(KH):
                            nc.tensor.matmul(
                                g_ps[:, hs], swg_sb[:, k, i2 * 128:(i2 + 1) * 128],
                                xT_bf[:, k, t0 + half * 512: t0 + (half + 1) * 512],
                                start=(k == 0), stop=(k == KH - 1),
                            )
                        for k in range(KH):
                            nc.tensor.matmul(
                                u_ps[:, hs], swu_sb[:, k, i2 * 128:(i2 + 1) * 128],
                                xT_bf[:, k, t0 + half * 512: t0 + (half + 1) * 512],
                                start=(k == 0), stop=(k == KH - 1),
                            )
                    _silu_mul(hsT[:, i2, :], g_ps, u_ps)
                # down projections
                for ts_ in range(NTS):
                    jj = ch * NTS + ts_
                    tsl = slice(ts_ * 128, (ts_ + 1) * 128)
                    o_ps = o_pool.tile([128, H], F32, tag="o")
                    for half in range(H // 512):
                        hs = slice(half * 512, (half + 1) * 512)
                        for i in range(KI):
                            nc.tensor.matmul(
                                o_ps[:, hs], hT[:, i, tsl], wd_sb[:, i, hs],
                                start=(i == 0), stop=(i == KI - 1),
                            )
                    o_sb = osb_pool.tile([128, H], F32, tag="osb")
                    nc.vector.tensor_scalar_mul(o_sb[:], o_ps[:], cwc_all[:, jj:jj + 1])
                    nc.scalar.dma_start(out=out_accs[ch][ts_ * 128:(ts_ + 1) * 128, :], in_=o_sb[:])
                    # shared down, accumulated into out_acc by DMA
                    o2_ps = o_pool.tile([128, H], F32, tag="o")
                    for half in range(H // 512):
                        hs = slice(half * 512, (half + 1) * 512)
                        for i2 in range(ISL // 128):
                            nc.tensor.matmul(
                                o2_ps[:, hs], hsT[:, i2, tsl], swd_sb[:, i2, hs],
                                start=(i2 == 0), stop=(i2 == ISL // 128 - 1),
                            )
                    o2_sb = osb_pool.tile([128, H], F32, tag="osb")
                    nc.vector.tensor_scalar_mul(o2_sb[:], o2_ps[:], sig_sb[:])
                    nc.gpsimd.dma_start(
                        out=out_accs[ch][ts_ * 128:(ts_ + 1) * 128, :], in_=o2_sb[:],
                        accum_op=OP.add,
                    )
                # this chunk of tokens is complete: reduce-scatter it now so the
                # collective overlaps the next chunk's compute
                nc.gpsimd.collective_compute(
                    "ReduceScatter",
                    OP.add,
                    ins=[out_accs[ch].opt()],
                    outs=[rs_outs[ch].opt()],
                    replica_groups=[list(range(N_CORES))],
                )
                nc.scalar.dma_start(
                    out=out_ext[ch * RSC:(ch + 1) * RSC, :], in_=rs_outs[ch][:]
                )


        # ---------------- aux loss reductions ----------------------------
        with (
            tc.tile_pool(name="stps", bufs=1, space="PSUM") as st_pool,
            tc.tile_pool(name="stsb", bufs=1) as st_sb_pool,
        ):
            stm = st_pool.tile([1, NT * E], F32)
            stp = st_pool.tile([1, NT * E], F32)
            stz = st_pool.tile([1, NT], F32)
            stl = st_pool.tile([1, NT * E], F32)
            nc.tensor.matmul(stm[:], ones[:], mask2[:].rearrange("p a b -> p (a b)"), start=True, stop=True)
            nc.tensor.matmul(stp[:], ones[:], probs[:].rearrange("p a b -> p (a b)"), start=True, stop=True)
            nc.tensor.matmul(stz[:], ones[:], zsq[:], start=True, stop=True)
            nc.tensor.matmul(stl[:], ones[:], pl[:].rearrange("p a b -> p (a b)"), start=True, stop=True)

            smv = st_sb_pool.tile([1, NT * E], F32)
            spv = st_sb_pool.tile([1, NT * E], F32)
            szv = st_sb_pool.tile([1, NT], F32)
            slv = st_sb_pool.tile([1, NT * E], F32)
            nc.scalar.activation(smv[:], stm[:], AF.Copy)
            nc.scalar.activation(spv[:], stp[:], AF.Copy)
            nc.scalar.activation(szv[:], stz[:], AF.Copy)
            nc.scalar.activation(slv[:], stl[:], AF.Copy)

            def _sum_over_tiles(dst, src):
                # src [1, NT*E] viewed as [1, E, NT] (strided) -> reduce X
                v = AP(src.tensor, src.offset, [list(src.ap[0]), [1, E], [E, NT]])
                nc.vector.tensor_reduce(dst, v, axis=mybir.AxisListType.X, op=OP.add)

            tpe = st_sb_pool.tile([1, E], F32)
            avg = st_sb_pool.tile([1, E], F32)
            _sum_over_tiles(tpe[:], smv[:])
            _sum_over_tiles(avg[:], spv[:])
            nc.vector.tensor_scalar_mul(tpe[:], tpe[:], 1.0 / (2 * T))
            nc.vector.tensor_scalar_mul(avg[:], avg[:], 1.0 / T)
            prod = st_sb_pool.tile([1, E], F32)
            nc.vector.tensor_tensor(prod[:], tpe[:], avg[:], OP.mult)
            lb = st_sb_pool.tile([1, 1], F32)
            nc.vector.tensor_reduce(lb[:], prod[:], axis=mybir.AxisListType.X, op=OP.add)
            nc.vector.tensor_scalar_mul(lb[:], lb[:], float(E))

            zt = st_sb_pool.tile([1, 1], F32)
            nc.vector.tensor_reduce(zt[:], szv[:], axis=mybir.AxisListType.X, op=OP.add)
            nc.vector.tensor_scalar_mul(zt[:], zt[:], 0.001 / T)

            el = st_sb_pool.tile([1, 1], F32)
            nc.vector.tensor_reduce(el[:], slv[:], axis=mybir.AxisListType.X, op=OP.add)
            # entropy = -sum/T ; el = (ln E - entropy) * 0.01 = sum*(0.01/T) + 0.01*lnE
            nc.vector.tensor_scalar(
                out=el[:], in0=el[:], scalar1=0.01 / T, scalar2=float(0.01 * np.log(E)),
                op0=OP.mult, op1=OP.add,
            )

            ug = st_sb_pool.tile([1, E], F32)
            nc.vector.tensor_scalar(out=ug[:], in0=tpe[:], scalar1=0.01, scalar2=None, op0=OP.is_gt)
            ul = st_sb_pool.tile([1, 1], F32)
            nc.vector.tensor_reduce(ul[:], ug[:], axis=mybir.AxisListType.X, op=OP.add)
            # util = (1 - usage/E) * 0.1 = -us*(0.1/E) + 0.1
            nc.vector.tensor_scalar(
                out=ul[:], in0=ul[:], scalar1=-0.1 / E, scalar2=0.1, op0=OP.mult, op1=OP.add,
            )

            auxv = st_sb_pool.tile([1, 1], F32)
            nc.vector.tensor_tensor(auxv[:], lb[:], zt[:], OP.add)
            nc.vector.tensor_tensor(auxv[:], auxv[:], el[:], OP.add)
            nc.vector.tensor_tensor(auxv[:], auxv[:], ul[:], OP.add)
            nc.vector.tensor_scalar(
                out=auxv[:], in0=auxv[:], scalar1=100.0, scalar2=0.0, op0=OP.min, op1=OP.max,
            )
            nc.sync.dma_start(out=aux_ext, in_=auxv[:])


    nc.compile()
    return nc


_CACHED = {}


def _get_program(T):
    if T not in _CACHED:
        _CACHED[T] = build_program(T)
    return _CACHED[T]


def make_in_maps(inputs: dict, T: int):
    x = np.ascontiguousarray(np.asarray(inputs["hidden_states"], dtype=np.float32).reshape(T, H))
    rwT = np.ascontiguousarray(np.asarray(inputs["router_w"], dtype=np.float32).T)
    wg = np.asarray(inputs["wg"], dtype=np.float32)
    wu = np.asarray(inputs["wu"], dtype=np.float32)
    wd = np.asarray(inputs["wd"], dtype=np.float32)
    swg = np.asarray(inputs["shared_wg"], dtype=np.float32)
    swu = np.asarray(inputs["shared_wu"], dtype=np.float32)
    swd = np.asarray(inputs["shared_wd"], dtype=np.float32)
    gate = float(np.asarray(inputs["shared_gate"]).reshape(-1)[0])
    in_maps = []
    for c in range(N_CORES):
        sel = np.zeros((128, E), dtype=np.float32)
        sel[:, c] = 1.0
        in_maps.append({
            "x": x,
            "rwT": rwT,
            "wg": np.ascontiguousarray(wg[c]),
            "wu": np.ascontiguousarray(wu[c]),
            "wd": np.ascontiguousarray(wd[c]),
            "swg": np.ascontiguousarray(swg[:, c * ISL:(c + 1) * ISL]),
            "swu": np.ascontiguousarray(swu[:, c * ISL:(c + 1) * ISL]),
            "swd": np.ascontiguousarray(swd[c * ISL:(c + 1) * ISL, :]),
            "sel": sel,
            "sgate": np.full((128, 1), gate, dtype=np.float32),
        })
    return in_maps


def assemble_out(results, T):
    # core c's "out_rs" rows are [chunk, 128] with global token
    # t = chunk*TC + c*128 + r (chunked ReduceScatter layout)
    TC = min(1024, T)
    NCH = T // TC
    stack = np.stack([results[c]["out_rs"] for c in range(N_CORES)], axis=0)
    stack = stack.reshape(N_CORES, NCH, TC // N_CORES, H).transpose(1, 0, 2, 3)
    return stack.reshape(T, H)


def kernel(**inputs):
    hs = np.asarray(inputs["hidden_states"])
    B, S, _ = hs.shape
    T = B * S
    nc = _get_program(T)
    in_maps = make_in_maps(inputs, T)
    res = run_bass_kernel_spmd(nc, in_maps, list(range(N_CORES)))
    out = assemble_out(res.results, T)
    aux = np.float32(res.results[0]["aux"].reshape(())[()])
    return out.reshape(B, S, H), aux


# revision 21
# speedup vs baseline: 1.0909x; 1.0909x over previous
"""Trainium2 Bass kernel for an 8-expert top-2 MoE layer with shared expert.

Sharding: expert-parallel. Each of the 8 cores owns one expert's FFN weights
plus a 1/8 slice (intermediate dim) of the shared expert. hidden_states and
the router are replicated; each core computes the router for all tokens in
fp32 (top-2 selection is precision-critical), the dense-masked FFN for its
own expert in bf16 (fp32 accumulation), and its shared-expert slice. Partial
outputs are summed with an on-device ReduceScatter; the host concatenates
the 8 token-slices. aux losses are computed redundantly on every core.

Self-contained: shapes hardcoded for B=2, S=2048, H=1024, I=2048, E=8.
"""

import sys

sys.path.insert(0, "/opt/trn_rl_repo")

from contextlib import ExitStack

import numpy as np

import concourse.bacc as bacc
import concourse.mybir as mybir
from concourse import masks, tile
from concourse.bass_types import AP
from concourse.bass_utils import run_bass_kernel_spmd

F32 = mybir.dt.float32
BF16 = mybir.dt.bfloat16
AF = mybir.ActivationFunctionType
OP = mybir.AluOpType

N_CORES = 8
H = 1024
I_DIM = 2048
E = 8
ISL = I_DIM // N_CORES  # shared-expert intermediate slice per core
KH = H // 128           # 8 k-tiles over H
KI = I_DIM // 128       # 16 k-tiles over I
EPS = 1e-6


def _bc_last(ap: AP, n: int) -> AP:
    """Broadcast an AP along a new innermost dim of size n (stride 0)."""
    return AP(ap.tensor, ap.offset, [list(x) for x in ap.ap] + [[0, n]])


def _bc_mid(ap: AP, n: int) -> AP:
    """[128, F] -> [128, n, F] broadcast on the middle dim (stride 0)."""
    a = [list(x) for x in ap.ap]
    return AP(ap.tensor, ap.offset, [a[0], [0, n], *a[1:]])


def build_program(T: int = 4096, tc_tokens: int = 1024, native_silu: bool = True):
    assert T % 1024 == 0 or T in (512,), T
    TC = min(tc_tokens, T)
    NT = T // 128          # token tiles
    NCH = T // TC          # ffn token chunks
    NTS = TC // 128        # token tiles per chunk
    TSL = T // N_CORES     # output slice per core after ReduceScatter

    nc = bacc.Bacc(
        "TRN2",
        target_bir_lowering=False,
        debug=False,
        enable_asserts=True,
        num_devices=N_CORES,
    )

    x_ext = nc.dram_tensor("x", [T, H], F32, kind="ExternalInput").ap()
    rwT_ext = nc.dram_tensor("rwT", [H, E], F32, kind="ExternalInput").ap()
    wg_ext = nc.dram_tensor("wg", [H, I_DIM], F32, kind="ExternalInput").ap()
    wu_ext = nc.dram_tensor("wu", [H, I_DIM], F32, kind="ExternalInput").ap()
    wd_ext = nc.dram_tensor("wd", [I_DIM, H], F32, kind="ExternalInput").ap()
    swg_ext = nc.dram_tensor("swg", [H, ISL], F32, kind="ExternalInput").ap()
    swu_ext = nc.dram_tensor("swu", [H, ISL], F32, kind="ExternalInput").ap()
    swd_ext = nc.dram_tensor("swd", [ISL, H], F32, kind="ExternalInput").ap()
    sel_ext = nc.dram_tensor("sel", [128, E], F32, kind="ExternalInput").ap()
    sgate_ext = nc.dram_tensor("sgate", [128, 1], F32, kind="ExternalInput").ap()

    out_ext = nc.dram_tensor("out_rs", [TSL, H], F32, kind="ExternalOutput").ap()
    aux_ext = nc.dram_tensor("aux", [1, 1], F32, kind="ExternalOutput").ap()

    with tile.TileContext(nc) as tc, ExitStack() as top:
        dram = top.enter_context(tc.tile_pool(name="dram", bufs=1, space="DRAM"))
        RSC = TC // N_CORES                   # rows per core per chunk
        out_accs = [dram.tile([TC, H], F32, name=f"acc{q}", tag=f"acc{q}") for q in range(NCH)]
        rs_outs = [dram.tile([RSC, H], F32, name=f"rsq{q}", tag=f"rsq{q}") for q in range(NCH)]
        wgc = dram.tile([KI, 128, KH * 128], BF16, name="wgc", tag="wgc")
        wuc = dram.tile([KI, 128, KH * 128], BF16, name="wuc", tag="wuc")

        const_pool = top.enter_context(tc.tile_pool(name="const", bufs=1))
        ident = const_pool.tile([128, 128], F32)
        masks.make_identity(nc, ident[:])
        ones = const_pool.tile([128, 1], F32)
        nc.vector.memset(ones[:], 1.0)
        rwT_sb = const_pool.tile([128, KH, E], F32)
        nc.sync.dma_start(out=rwT_sb[:], in_=rwT_ext.rearrange("(k p) e -> p k e", p=128))
        sel_sb = const_pool.tile([128, E], F32)
        nc.sync.dma_start(out=sel_sb[:], in_=sel_ext)
        sig_sb = const_pool.tile([128, 1], F32)
        sg_in = const_pool.tile([128, 1], F32)
        nc.sync.dma_start(out=sg_in[:], in_=sgate_ext)
        nc.scalar.activation(sig_sb[:], sg_in[:], AF.Sigmoid)

        # Resident activations / weights
        big_pool = top.enter_context(tc.tile_pool(name="resident", bufs=1))
        xT_bf = big_pool.tile([128, KH, T], BF16)          # x^T, bf16
        wd_sb = big_pool.tile([128, KI, H], BF16)          # wd tiles [i_k][i_p, h]
        swg_sb = big_pool.tile([128, KH, ISL], BF16)
        swu_sb = big_pool.tile([128, KH, ISL], BF16)
        swd_sb = big_pool.tile([128, ISL // 128, H], BF16)
        nc.gpsimd.dma_start(out=wd_sb[:], in_=wd_ext.rearrange("(k p) h -> p k h", p=128))
        nc.gpsimd.dma_start(out=swg_sb[:], in_=swg_ext.rearrange("(k p) i -> p k i", p=128))
        nc.gpsimd.dma_start(out=swu_sb[:], in_=swu_ext.rearrange("(k p) i -> p k i", p=128))
        nc.gpsimd.dma_start(out=swd_sb[:], in_=swd_ext.rearrange("(k p) h -> p k h", p=128))

        # Router tensors (fp32), persistent until aux is finalized
        rt_pool = top.enter_context(tc.tile_pool(name="router", bufs=1))
        logits_all = rt_pool.tile([128, NT, E], F32)
        probs = rt_pool.tile([128, NT, E], F32)
        mask2 = rt_pool.tile([128, NT, E], F32)
        pl = rt_pool.tile([128, NT, E], F32)
        zsq = rt_pool.tile([128, NT], F32)
        cwc_all = rt_pool.tile([128, NT], F32)

        # ---------------- Phase A: x load + transpose + router logits ----
        with (
            tc.tile_pool(name="xin", bufs=3) as xin_pool,
            tc.tile_pool(name="xtT", bufs=2) as xtT_pool,
            tc.tile_pool(name="trps", bufs=2, space="PSUM") as tr_pool,
            tc.tile_pool(name="lgps", bufs=2, space="PSUM") as lg_pool,
        ):
            for j in range(NT):
                xt = xin_pool.tile([128, H], F32)
                nc.sync.dma_start(out=xt[:], in_=x_ext[j * 128:(j + 1) * 128, :])
                tr_ps = tr_pool.tile([128, H], F32)
                for k in range(KH):
                    nc.tensor.transpose(
                        tr_ps[:, k * 128:(k + 1) * 128],
                        xt[:, k * 128:(k + 1) * 128],
                        ident[:],
                    )
                xtT = xtT_pool.tile([128, H], F32)
                nc.scalar.activation(xtT[:], tr_ps[:], AF.Copy)
                nc.vector.tensor_copy(
                    xT_bf[:, :, j * 128:(j + 1) * 128],
                    xtT[:].rearrange("p (k t) -> p k t", k=KH),
                )
                lg_ps = lg_pool.tile([128, E], F32)
                for k in range(KH):
                    nc.tensor.matmul(
                        lg_ps[:],
                        xtT[:, k * 128:(k + 1) * 128],
                        rwT_sb[:, k, :],
                        start=(k == 0),
                        stop=(k == KH - 1),
                    )
                nc.vector.tensor_scalar(
                    out=logits_all[:, j, :], in0=lg_ps[:],
                    scalar1=50.0, scalar2=-50.0, op0=OP.min, op1=OP.max,
                )

        # ---------------- Router math (batched over all tokens) ----------
        with tc.tile_pool(name="rtmp", bufs=1) as rt:
            mx = rt.tile([128, NT], F32)
            nc.vector.tensor_reduce(mx[:], logits_all[:], axis=mybir.AxisListType.X, op=OP.max)
            shifted = rt.tile([128, NT, E], F32)
            nc.vector.tensor_tensor(shifted[:], logits_all[:], _bc_last(mx[:], E), OP.subtract)
            exps = rt.tile([128, NT, E], F32)
            nc.scalar.activation(exps[:], shifted[:], AF.Exp)
            sums = rt.tile([128, NT], F32)
            nc.vector.tensor_reduce(sums[:], exps[:], axis=mybir.AxisListType.X, op=OP.add)
            rcp = rt.tile([128, NT], F32)
            nc.vector.reciprocal(rcp[:], sums[:])
            nc.vector.tensor_tensor(probs[:], exps[:], _bc_last(rcp[:], E), OP.mult)
            # z-loss: lse = ln(sum) + max, squared
            lns = rt.tile([128, NT], F32)
            nc.scalar.activation(lns[:], sums[:], AF.Ln)
            lse = rt.tile([128, NT], F32)
            nc.vector.tensor_tensor(lse[:], lns[:], mx[:], OP.add)
            nc.vector.tensor_tensor(zsq[:], lse[:], lse[:], OP.mult)
            # top-2 selection on LOGITS (same order as probs; avoids any
            # dependence of the selection on exp-LUT rounding)
            m1 = rt.tile([128, NT], F32)
            nc.vector.tensor_reduce(m1[:], logits_all[:], axis=mybir.AxisListType.X, op=OP.max)
            eq1 = rt.tile([128, NT, E], F32)
            nc.vector.tensor_tensor(eq1[:], logits_all[:], _bc_last(m1[:], E), OP.is_ge)
            pm = rt.tile([128, NT, E], F32)
            nc.vector.tensor_scalar_mul(pm[:], eq1[:], 1.0e9)
            nc.vector.tensor_tensor(pm[:], logits_all[:], pm[:], OP.subtract)
            m2 = rt.tile([128, NT], F32)
            nc.vector.tensor_reduce(m2[:], pm[:], axis=mybir.AxisListType.X, op=OP.max)
            nc.vector.tensor_tensor(mask2[:], logits_all[:], _bc_last(m2[:], E), OP.is_ge)
            # cw = probs * mask2 / (sum of selected probs + EPS)
            cw = rt.tile([128, NT, E], F32)
            nc.vector.tensor_tensor(cw[:], probs[:], mask2[:], OP.mult)
            den = rt.tile([128, NT], F32)
            nc.vector.tensor_reduce(den[:], cw[:], axis=mybir.AxisListType.X, op=OP.add)
            nc.vector.tensor_scalar_add(den[:], den[:], EPS)
            rd = rt.tile([128, NT], F32)
            nc.vector.reciprocal(rd[:], den[:])
            nc.vector.tensor_tensor(cw[:], cw[:], _bc_last(rd[:], E), OP.mult)
            cwsel = rt.tile([128, NT, E], F32)
            nc.vector.tensor_tensor(cwsel[:], cw[:], _bc_mid(sel_sb[:], NT), OP.mult)
            nc.vector.tensor_reduce(cwc_all[:], cwsel[:], axis=mybir.AxisListType.X, op=OP.add)
            # entropy terms
            psafe = rt.tile([128, NT, E], F32)
            nc.vector.tensor_scalar_max(psafe[:], probs[:], EPS)
            lp = rt.tile([128, NT, E], F32)
            nc.scalar.activation(lp[:], psafe[:], AF.Ln)
            nc.vector.tensor_tensor(pl[:], psafe[:], lp[:], OP.mult)

        # ---------------- aux loss reductions ----------------------------
        with (
            tc.tile_pool(name="stps", bufs=1, space="PSUM") as st_pool,
            tc.tile_pool(name="stsb", bufs=1) as st_sb_pool,
        ):
            stm = st_pool.tile([1, NT * E], F32)
            stp = st_pool.tile([1, NT * E], F32)
            stz = st_pool.tile([1, NT], F32)
            stl = st_pool.tile([1, NT * E], F32)
            nc.tensor.matmul(stm[:], ones[:], mask2[:].rearrange("p a b -> p (a b)"), start=True, stop=True)
            nc.tensor.matmul(stp[:], ones[:], probs[:].rearrange("p a b -> p (a b)"), start=True, stop=True)
            nc.tensor.matmul(stz[:], ones[:], zsq[:], start=True, stop=True)
            nc.tensor.matmul(stl[:], ones[:], pl[:].rearrange("p a b -> p (a b)"), start=True, stop=True)

            smv = st_sb_pool.tile([1, NT * E], F32)
            spv = st_sb_pool.tile([1, NT * E], F32)
            szv = st_sb_pool.tile([1, NT], F32)
            slv = st_sb_pool.tile([1, NT * E], F32)
            nc.scalar.activation(smv[:], stm[:], AF.Copy)
            nc.scalar.activation(spv[:], stp[:], AF.Copy)
            nc.scalar.activation(szv[:], stz[:], AF.Copy)
            nc.scalar.activation(slv[:], stl[:], AF.Copy)

            def _sum_over_tiles(dst, src):
                # src [1, NT*E] viewed as [1, E, NT] (strided) -> reduce X
                v = AP(src.tensor, src.offset, [list(src.ap[0]), [1, E], [E, NT]])
                nc.vector.tensor_reduce(dst, v, axis=mybir.AxisListType.X, op=OP.add)

            tpe = st_sb_pool.tile([1, E], F32)
            avg = st_sb_pool.tile([1, E], F32)
            _sum_over_tiles(tpe[:], smv[:])
            _sum_over_tiles(avg[:], spv[:])
            nc.vector.tensor_scalar_mul(tpe[:], tpe[:], 1.0 / (2 * T))
            nc.vector.tensor_scalar_mul(avg[:], avg[:], 1.0 / T)
            prod = st_sb_pool.tile([1, E], F32)
            nc.vector.tensor_tensor(prod[:], tpe[:], avg[:], OP.mult)
            lb = st_sb_pool.tile([1, 1], F32)
            nc.vector.tensor_reduce(lb[:], prod[:], axis=mybir.AxisListType.X, op=OP.add)
            nc.vector.tensor_scalar_mul(lb[:], lb[:], float(E))

            zt = st_sb_pool.tile([1, 1], F32)
            nc.vector.tensor_reduce(zt[:], szv[:], axis=mybir.AxisListType.X, op=OP.add)
            nc.vector.tensor_scalar_mul(zt[:], zt[:], 0.001 / T)

            el = st_sb_pool.tile([1, 1], F32)
            nc.vector.tensor_reduce(el[:], slv[:], axis=mybir.AxisListType.X, op=OP.add)
            # entropy = -sum/T ; el = (ln E - entropy) * 0.01 = sum*(0.01/T) + 0.01*lnE
            nc.vector.tensor_scalar(
                out=el[:], in0=el[:], scalar1=0.01 / T, scalar2=float(0.01 * np.log(E)),
                op0=OP.mult, op1=OP.add,
            )

            ug = st_sb_pool.tile([1, E], F32)
            nc.vector.tensor_scalar(out=ug[:], in0=tpe[:], scalar1=0.01, scalar2=None, op0=OP.is_gt)
            ul = st_sb_pool.tile([1, 1], F32)
            nc.vector.tensor_reduce(ul[:], ug[:], axis=mybir.AxisListType.X, op=OP.add)
            # util = (1 - usage/E) * 0.1 = -us*(0.1/E) + 0.1
            nc.vector.tensor_scalar(
                out=ul[:], in0=ul[:], scalar1=-0.1 / E, scalar2=0.1, op0=OP.mult, op1=OP.add,
            )

            auxv = st_sb_pool.tile([1, 1], F32)
            nc.vector.tensor_tensor(auxv[:], lb[:], zt[:], OP.add)
            nc.vector.tensor_tensor(auxv[:], auxv[:], el[:], OP.add)
            nc.vector.tensor_tensor(auxv[:], auxv[:], ul[:], OP.add)
            nc.vector.tensor_scalar(
                out=auxv[:], in0=auxv[:], scalar1=100.0, scalar2=0.0, op0=OP.min, op1=OP.max,
            )
            nc.sync.dma_start(out=aux_ext, in_=auxv[:])

        # ---------------- Phase B: routed expert FFN (dense, masked) -----
        # ---------------- Phase C: shared expert slice --------------------
        with (
            tc.tile_pool(name="wstream", bufs=3) as wstream,
            tc.tile_pool(name="gact", bufs=2) as gact_pool,
            tc.tile_pool(name="hT", bufs=1) as hT_pool,
            tc.tile_pool(name="osb", bufs=3) as osb_pool,
            tc.tile_pool(name="gps", bufs=1, space="PSUM") as g_pool,
            tc.tile_pool(name="ups", bufs=1, space="PSUM") as u_pool,
            tc.tile_pool(name="ops", bufs=2, space="PSUM") as o_pool,
        ):
            hT = hT_pool.tile([128, KI, TC], BF16, tag="hT")
            hsT = hT_pool.tile([128, ISL // 128, TC], BF16, tag="hsT")

            def _silu_mul(dst, g_ps, u_ps):
                ga = gact_pool.tile([128, TC], BF16, tag="ga")
                if native_silu:
                    nc.scalar.activation(ga[:], g_ps[:], AF.Silu)
                else:
                    sgm = gact_pool.tile([128, TC], F32, tag="sgm")
                    nc.scalar.activation(sgm[:], g_ps[:], AF.Sigmoid)
                    nc.vector.tensor_tensor(ga[:], sgm[:], g_ps[:], OP.mult)
                nc.vector.tensor_tensor(dst, ga[:], u_ps[:], OP.mult)
            for ch in range(NCH):
                t0 = ch * TC
                # gate/up for the routed expert
                for i in range(KI):
                    wg_sb = wstream.tile([128, KH, 128], BF16, tag="wg")
                    wu_sb = wstream.tile([128, KH, 128], BF16, tag="wu")
                    if ch == 0:
                        # first pass: cast-load fp32 -> bf16 (SWDGE), then cache
                        # the bf16 tiles in DRAM so later chunks reload via
                        # HWDGE and keep the gpsimd queue free for collectives
                        nc.gpsimd.dma_start(
                            out=wg_sb[:],
                            in_=AP(wg_ext.tensor, i * 128, [[I_DIM, 128], [128 * I_DIM, KH], [1, 128]]),
                        )
                        nc.gpsimd.dma_start(
                            out=wu_sb[:],
                            in_=AP(wu_ext.tensor, i * 128, [[I_DIM, 128], [128 * I_DIM, KH], [1, 128]]),
                        )
                        if NCH > 1:
                            nc.scalar.dma_start(out=wgc[i], in_=wg_sb[:])
                            nc.scalar.dma_start(out=wuc[i], in_=wu_sb[:])
                    else:
                        nc.sync.dma_start(out=wg_sb[:].rearrange("p a b -> p (a b)"), in_=wgc[i])
                        nc.sync.dma_start(out=wu_sb[:].rearrange("p a b -> p (a b)"), in_=wuc[i])
                    g_ps = g_pool.tile([128, TC], F32, tag="g")
                    u_ps = u_pool.tile([128, TC], F32, tag="u")
                    for half in range(TC // 512):
                        hs = slice(half * 512, (half + 1) * 512)
                        for k in range(KH):
                            nc.tensor.matmul(
                                g_ps[:, hs], wg_sb[:, k, :],
                                xT_bf[:, k, t0 + half * 512: t0 + (half + 1) * 512],
                                start=(k == 0), stop=(k == KH - 1),
                            )
                        for k in range(KH):
                            nc.tensor.matmul(
                                u_ps[:, hs], wu_sb[:, k, :],
                                xT_bf[:, k, t0 + half * 512: t0 + (half + 1) * 512],
                                start=(k == 0), stop=(k == KH - 1),
                            )
                    _silu_mul(hT[:, i, :], g_ps, u_ps)
                # gate/up for the shared expert slice
                for i2 in range(ISL // 128):
                    g_ps = g_pool.tile([128, TC], F32, tag="g")
                    u_ps = u_pool.tile([128, TC], F32, tag="u")
                    for half in range(TC // 512):
                        hs = slice(half * 512, (half + 1) * 512)
                        for k in range(KH):
                            nc.tensor.matmul(
                                g_ps[:, hs], swg_sb[:, k, i2 * 128:(i2 + 1) * 128],
                                xT_bf[:, k, t0 + half * 512: t0 + (half + 1) * 512],
                                start=(k == 0), stop=(k == KH - 1),
                            )
                        for k in range(KH):
                            nc.tensor.matmul(
                                u_ps[:, hs], swu_sb[:, k, i2 * 128:(i2 + 1) * 128],
                                xT_bf[:, k, t0 + half * 512: t0 + (half + 1) * 512],
                                start=(k == 0), stop=(k == KH - 1),
                            )
                    _silu_mul(hsT[:, i2, :], g_ps, u_ps)
                # down projections
                for ts_ in range(NTS):
                    jj = ch * NTS + ts_
                    tsl = slice(ts_ * 128, (ts_ + 1) * 128)
                    o_ps = o_pool.tile([128, H], F32, tag="o")
                    for half in range(H // 512):
                        hs = slice(half * 512, (half + 1) * 512)
                        for i in range(KI):
                            nc.tensor.matmul(
                                o_ps[:, hs], hT[:, i, tsl], wd_sb[:, i, hs],
                                start=(i == 0), stop=(i == KI - 1),
                            )
                    o_sb = osb_pool.tile([128, H], F32, tag="osb")
                    nc.vector.tensor_scalar_mul(o_sb[:], o_ps[:], cwc_all[:, jj:jj + 1])
                    nc.sync.dma_start(out=out_accs[ch][ts_ * 128:(ts_ + 1) * 128, :], in_=o_sb[:])
                    # shared down, accumulated into out_acc by DMA
                    o2_ps = o_pool.tile([128, H], F32, tag="o")
                    for half in range(H // 512):
                        hs = slice(half * 512, (half + 1) * 512)
                        for i2 in range(ISL // 128):
                            nc.tensor.matmul(
                                o2_ps[:, hs], hsT[:, i2, tsl], swd_sb[:, i2, hs],
                                start=(i2 == 0), stop=(i2 == ISL // 128 - 1),
                            )
                    o2_sb = osb_pool.tile([128, H], F32, tag="osb")
                    nc.vector.tensor_scalar_mul(o2_sb[:], o2_ps[:], sig_sb[:])
                    nc.gpsimd.dma_start(
                        out=out_accs[ch][ts_ * 128:(ts_ + 1) * 128, :], in_=o2_sb[:],
                        accum_op=OP.add,
                    )
                # this chunk of tokens is complete: reduce-scatter it now so the
                # collective overlaps the next chunk's compute
                nc.gpsimd.collective_compute(
                    "ReduceScatter",
                    OP.add,
                    ins=[out_accs[ch].opt()],
                    outs=[rs_outs[ch].opt()],
                    replica_groups=[list(range(N_CORES))],
                )
                nc.scalar.dma_start(
                    out=out_ext[ch * RSC:(ch + 1) * RSC, :], in_=rs_outs[ch][:]
                )


    nc.compile()
    return nc


_CACHED = {}


def _get_program(T):
    if T not in _CACHED:
        _CACHED[T] = build_program(T)
    return _CACHED[T]


def make_in_maps(inputs: dict, T: int):
    x = np.ascontiguousarray(np.asarray(inputs["hidden_states"], dtype=np.float32).reshape(T, H))
    rwT = np.ascontiguousarray(np.asarray(inputs["router_w"], dtype=np.float32).T)
    wg = np.asarray(inputs["wg"], dtype=np.float32)
    wu = np.asarray(inputs["wu"], dtype=np.float32)
    wd = np.asarray(inputs["wd"], dtype=np.float32)
    swg = np.asarray(inputs["shared_wg"], dtype=np.float32)
    swu = np.asarray(inputs["shared_wu"], dtype=np.float32)
    swd = np.asarray(inputs["shared_wd"], dtype=np.float32)
    gate = float(np.asarray(inputs["shared_gate"]).reshape(-1)[0])
    in_maps = []
    for c in range(N_CORES):
        sel = np.zeros((128, E), dtype=np.float32)
        sel[:, c] = 1.0
        in_maps.append({
            "x": x,
            "rwT": rwT,
            "wg": np.ascontiguousarray(wg[c]),
            "wu": np.ascontiguousarray(wu[c]),
            "wd": np.ascontiguousarray(wd[c]),
            "swg": np.ascontiguousarray(swg[:, c * ISL:(c + 1) * ISL]),
            "swu": np.ascontiguousarray(swu[:, c * ISL:(c + 1) * ISL]),
            "swd": np.ascontiguousarray(swd[c * ISL:(c + 1) * ISL, :]),
            "sel": sel,
            "sgate": np.full((128, 1), gate, dtype=np.float32),
        })
    return in_maps


def assemble_out(results, T):
    # core c's "out_rs" rows are [chunk, 128] with global token
    # t = chunk*TC + c*128 + r (chunked ReduceScatter layout)
    TC = min(1024, T)
    NCH = T // TC
    stack = np.stack([results[c]["out_rs"] for c in range(N_CORES)], axis=0)
    stack = stack.reshape(N_CORES, NCH, TC // N_CORES, H).transpose(1, 0, 2, 3)
    return stack.reshape(T, H)


def kernel(**inputs):
    hs = np.asarray(inputs["hidden_states"])
    B, S, _ = hs.shape
    T = B * S
    nc = _get_program(T)
    in_maps = make_in_maps(inputs, T)
    res = run_bass_kernel_spmd(nc, in_maps, list(range(N_CORES)))
    out = assemble_out(res.results, T)
    aux = np.float32(res.results[0]["aux"].reshape(())[()])
    return out.reshape(B, S, H), aux


# revision 22
# speedup vs baseline: 1.0925x; 1.0015x over previous
"""Trainium2 Bass kernel for an 8-expert top-2 MoE layer with shared expert.

Sharding: expert-parallel. Each of the 8 cores owns one expert's FFN weights
plus a 1/8 slice (intermediate dim) of the shared expert. hidden_states and
the router are replicated; each core computes the router for all tokens in
fp32 (top-2 selection is precision-critical), the dense-masked FFN for its
own expert in bf16 (fp32 accumulation), and its shared-expert slice. Partial
outputs are summed with an on-device ReduceScatter; the host concatenates
the 8 token-slices. aux losses are computed redundantly on every core.

Self-contained: shapes hardcoded for B=2, S=2048, H=1024, I=2048, E=8.
"""

import sys

sys.path.insert(0, "/opt/trn_rl_repo")

from contextlib import ExitStack

import numpy as np

import concourse.bacc as bacc
import concourse.mybir as mybir
from concourse import masks, tile
from concourse.bass_types import AP
from concourse.bass_utils import run_bass_kernel_spmd

F32 = mybir.dt.float32
BF16 = mybir.dt.bfloat16
AF = mybir.ActivationFunctionType
OP = mybir.AluOpType

N_CORES = 8
H = 1024
I_DIM = 2048
E = 8
ISL = I_DIM // N_CORES  # shared-expert intermediate slice per core
KH = H // 128           # 8 k-tiles over H
KI = I_DIM // 128       # 16 k-tiles over I
EPS = 1e-6


def _bc_last(ap: AP, n: int) -> AP:
    """Broadcast an AP along a new innermost dim of size n (stride 0)."""
    return AP(ap.tensor, ap.offset, [list(x) for x in ap.ap] + [[0, n]])


def _bc_mid(ap: AP, n: int) -> AP:
    """[128, F] -> [128, n, F] broadcast on the middle dim (stride 0)."""
    a = [list(x) for x in ap.ap]
    return AP(ap.tensor, ap.offset, [a[0], [0, n], *a[1:]])


def build_program(T: int = 4096, tc_tokens: int = 1024, native_silu: bool = True):
    assert T % 1024 == 0 or T in (512,), T
    TC = min(tc_tokens, T)
    NT = T // 128          # token tiles
    NCH = T // TC          # ffn token chunks
    NTS = TC // 128        # token tiles per chunk
    TSL = T // N_CORES     # output slice per core after ReduceScatter

    nc = bacc.Bacc(
        "TRN2",
        target_bir_lowering=False,
        debug=False,
        enable_asserts=True,
        num_devices=N_CORES,
    )

    x_ext = nc.dram_tensor("x", [T, H], F32, kind="ExternalInput").ap()
    rwT_ext = nc.dram_tensor("rwT", [H, E], F32, kind="ExternalInput").ap()
    wg_ext = nc.dram_tensor("wg", [H, I_DIM], F32, kind="ExternalInput").ap()
    wu_ext = nc.dram_tensor("wu", [H, I_DIM], F32, kind="ExternalInput").ap()
    wd_ext = nc.dram_tensor("wd", [I_DIM, H], F32, kind="ExternalInput").ap()
    swg_ext = nc.dram_tensor("swg", [H, ISL], F32, kind="ExternalInput").ap()
    swu_ext = nc.dram_tensor("swu", [H, ISL], F32, kind="ExternalInput").ap()
    swd_ext = nc.dram_tensor("swd", [ISL, H], F32, kind="ExternalInput").ap()
    sel_ext = nc.dram_tensor("sel", [128, E], F32, kind="ExternalInput").ap()
    sgate_ext = nc.dram_tensor("sgate", [128, 1], F32, kind="ExternalInput").ap()

    out_ext = nc.dram_tensor("out_rs", [TSL, H], F32, kind="ExternalOutput").ap()
    aux_ext = nc.dram_tensor("aux", [1, 1], F32, kind="ExternalOutput").ap()

    with tile.TileContext(nc) as tc, ExitStack() as top:
        dram = top.enter_context(tc.tile_pool(name="dram", bufs=1, space="DRAM"))
        RSC = TC // N_CORES                   # rows per core per chunk
        out_accs = [dram.tile([TC, H], F32, name=f"acc{q}", tag=f"acc{q}") for q in range(NCH)]
        rs_outs = [dram.tile([RSC, H], F32, name=f"rsq{q}", tag=f"rsq{q}") for q in range(NCH)]
        wgc = dram.tile([KI, 128, KH * 128], BF16, name="wgc", tag="wgc")
        wuc = dram.tile([KI, 128, KH * 128], BF16, name="wuc", tag="wuc")

        const_pool = top.enter_context(tc.tile_pool(name="const", bufs=1))
        ident = const_pool.tile([128, 128], F32)
        masks.make_identity(nc, ident[:])
        ones = const_pool.tile([128, 1], F32)
        nc.vector.memset(ones[:], 1.0)
        rwT_sb = const_pool.tile([128, KH, E], F32)
        nc.sync.dma_start(out=rwT_sb[:], in_=rwT_ext.rearrange("(k p) e -> p k e", p=128))
        sel_sb = const_pool.tile([128, E], F32)
        nc.sync.dma_start(out=sel_sb[:], in_=sel_ext)
        sig_sb = const_pool.tile([128, 1], F32)
        sg_in = const_pool.tile([128, 1], F32)
        nc.sync.dma_start(out=sg_in[:], in_=sgate_ext)
        nc.scalar.activation(sig_sb[:], sg_in[:], AF.Sigmoid)

        # Resident activations / weights
        big_pool = top.enter_context(tc.tile_pool(name="resident", bufs=1))
        xT_bf = big_pool.tile([128, KH, T], BF16)          # x^T, bf16
        wd_sb = big_pool.tile([128, KI, H], BF16)          # wd tiles [i_k][i_p, h]
        swg_sb = big_pool.tile([128, KH, ISL], BF16)
        swu_sb = big_pool.tile([128, KH, ISL], BF16)
        swd_sb = big_pool.tile([128, ISL // 128, H], BF16)

        # Router tensors (fp32), persistent until aux is finalized
        rt_pool = top.enter_context(tc.tile_pool(name="router", bufs=1))
        logits_all = rt_pool.tile([128, NT, E], F32)
        probs = rt_pool.tile([128, NT, E], F32)
        mask2 = rt_pool.tile([128, NT, E], F32)
        pl = rt_pool.tile([128, NT, E], F32)
        zsq = rt_pool.tile([128, NT], F32)
        cwc_all = rt_pool.tile([128, NT], F32)

        # ---------------- Phase A: x load + transpose + router logits ----
        with (
            tc.tile_pool(name="xin", bufs=3) as xin_pool,
            tc.tile_pool(name="xtT", bufs=2) as xtT_pool,
            tc.tile_pool(name="trps", bufs=2, space="PSUM") as tr_pool,
            tc.tile_pool(name="lgps", bufs=2, space="PSUM") as lg_pool,
        ):
            for j in range(NT):
                xt = xin_pool.tile([128, H], F32)
                nc.sync.dma_start(out=xt[:], in_=x_ext[j * 128:(j + 1) * 128, :])
                tr_ps = tr_pool.tile([128, H], F32)
                for k in range(KH):
                    nc.tensor.transpose(
                        tr_ps[:, k * 128:(k + 1) * 128],
                        xt[:, k * 128:(k + 1) * 128],
                        ident[:],
                    )
                xtT = xtT_pool.tile([128, H], F32)
                nc.scalar.activation(xtT[:], tr_ps[:], AF.Copy)
                nc.vector.tensor_copy(
                    xT_bf[:, :, j * 128:(j + 1) * 128],
                    xtT[:].rearrange("p (k t) -> p k t", k=KH),
                )
                lg_ps = lg_pool.tile([128, E], F32)
                for k in range(KH):
                    nc.tensor.matmul(
                        lg_ps[:],
                        xtT[:, k * 128:(k + 1) * 128],
                        rwT_sb[:, k, :],
                        start=(k == 0),
                        stop=(k == KH - 1),
                    )
                nc.vector.tensor_scalar(
                    out=logits_all[:, j, :], in0=lg_ps[:],
                    scalar1=50.0, scalar2=-50.0, op0=OP.min, op1=OP.max,
                )

        # ---------------- Router math (batched over all tokens) ----------
        with tc.tile_pool(name="rtmp", bufs=1) as rt:
            mx = rt.tile([128, NT], F32)
            nc.vector.tensor_reduce(mx[:], logits_all[:], axis=mybir.AxisListType.X, op=OP.max)
            shifted = rt.tile([128, NT, E], F32)
            nc.vector.tensor_tensor(shifted[:], logits_all[:], _bc_last(mx[:], E), OP.subtract)
            exps = rt.tile([128, NT, E], F32)
            nc.scalar.activation(exps[:], shifted[:], AF.Exp)
            sums = rt.tile([128, NT], F32)
            nc.vector.tensor_reduce(sums[:], exps[:], axis=mybir.AxisListType.X, op=OP.add)
            rcp = rt.tile([128, NT], F32)
            nc.vector.reciprocal(rcp[:], sums[:])
            nc.vector.tensor_tensor(probs[:], exps[:], _bc_last(rcp[:], E), OP.mult)
            # z-loss: lse = ln(sum) + max, squared
            lns = rt.tile([128, NT], F32)
            nc.scalar.activation(lns[:], sums[:], AF.Ln)
            lse = rt.tile([128, NT], F32)
            nc.vector.tensor_tensor(lse[:], lns[:], mx[:], OP.add)
            nc.vector.tensor_tensor(zsq[:], lse[:], lse[:], OP.mult)
            # top-2 selection on LOGITS (same order as probs; avoids any
            # dependence of the selection on exp-LUT rounding)
            m1 = rt.tile([128, NT], F32)
            nc.vector.tensor_reduce(m1[:], logits_all[:], axis=mybir.AxisListType.X, op=OP.max)
            eq1 = rt.tile([128, NT, E], F32)
            nc.vector.tensor_tensor(eq1[:], logits_all[:], _bc_last(m1[:], E), OP.is_ge)
            pm = rt.tile([128, NT, E], F32)
            nc.vector.tensor_scalar_mul(pm[:], eq1[:], 1.0e9)
            nc.vector.tensor_tensor(pm[:], logits_all[:], pm[:], OP.subtract)
            m2 = rt.tile([128, NT], F32)
            nc.vector.tensor_reduce(m2[:], pm[:], axis=mybir.AxisListType.X, op=OP.max)
            nc.vector.tensor_tensor(mask2[:], logits_all[:], _bc_last(m2[:], E), OP.is_ge)
            # cw = probs * mask2 / (sum of selected probs + EPS)
            cw = rt.tile([128, NT, E], F32)
            nc.vector.tensor_tensor(cw[:], probs[:], mask2[:], OP.mult)
            den = rt.tile([128, NT], F32)
            nc.vector.tensor_reduce(den[:], cw[:], axis=mybir.AxisListType.X, op=OP.add)
            nc.vector.tensor_scalar_add(den[:], den[:], EPS)
            rd = rt.tile([128, NT], F32)
            nc.vector.reciprocal(rd[:], den[:])
            nc.vector.tensor_tensor(cw[:], cw[:], _bc_last(rd[:], E), OP.mult)
            cwsel = rt.tile([128, NT, E], F32)
            nc.vector.tensor_tensor(cwsel[:], cw[:], _bc_mid(sel_sb[:], NT), OP.mult)
            nc.vector.tensor_reduce(cwc_all[:], cwsel[:], axis=mybir.AxisListType.X, op=OP.add)
            # entropy terms
            psafe = rt.tile([128, NT, E], F32)
            nc.vector.tensor_scalar_max(psafe[:], probs[:], EPS)
            lp = rt.tile([128, NT, E], F32)
            nc.scalar.activation(lp[:], psafe[:], AF.Ln)
            nc.vector.tensor_tensor(pl[:], psafe[:], lp[:], OP.mult)

        # ---------------- aux loss reductions ----------------------------
        with (
            tc.tile_pool(name="stps", bufs=1, space="PSUM") as st_pool,
            tc.tile_pool(name="stsb", bufs=1) as st_sb_pool,
        ):
            stm = st_pool.tile([1, NT * E], F32)
            stp = st_pool.tile([1, NT * E], F32)
            stz = st_pool.tile([1, NT], F32)
            stl = st_pool.tile([1, NT * E], F32)
            nc.tensor.matmul(stm[:], ones[:], mask2[:].rearrange("p a b -> p (a b)"), start=True, stop=True)
            nc.tensor.matmul(stp[:], ones[:], probs[:].rearrange("p a b -> p (a b)"), start=True, stop=True)
            nc.tensor.matmul(stz[:], ones[:], zsq[:], start=True, stop=True)
            nc.tensor.matmul(stl[:], ones[:], pl[:].rearrange("p a b -> p (a b)"), start=True, stop=True)

            smv = st_sb_pool.tile([1, NT * E], F32)
            spv = st_sb_pool.tile([1, NT * E], F32)
            szv = st_sb_pool.tile([1, NT], F32)
            slv = st_sb_pool.tile([1, NT * E], F32)
            nc.scalar.activation(smv[:], stm[:], AF.Copy)
            nc.scalar.activation(spv[:], stp[:], AF.Copy)
            nc.scalar.activation(szv[:], stz[:], AF.Copy)
            nc.scalar.activation(slv[:], stl[:], AF.Copy)

            def _sum_over_tiles(dst, src):
                # src [1, NT*E] viewed as [1, E, NT] (strided) -> reduce X
                v = AP(src.tensor, src.offset, [list(src.ap[0]), [1, E], [E, NT]])
                nc.vector.tensor_reduce(dst, v, axis=mybir.AxisListType.X, op=OP.add)

            tpe = st_sb_pool.tile([1, E], F32)
            avg = st_sb_pool.tile([1, E], F32)
            _sum_over_tiles(tpe[:], smv[:])
            _sum_over_tiles(avg[:], spv[:])
            nc.vector.tensor_scalar_mul(tpe[:], tpe[:], 1.0 / (2 * T))
            nc.vector.tensor_scalar_mul(avg[:], avg[:], 1.0 / T)
            prod = st_sb_pool.tile([1, E], F32)
            nc.vector.tensor_tensor(prod[:], tpe[:], avg[:], OP.mult)
            lb = st_sb_pool.tile([1, 1], F32)
            nc.vector.tensor_reduce(lb[:], prod[:], axis=mybir.AxisListType.X, op=OP.add)
            nc.vector.tensor_scalar_mul(lb[:], lb[:], float(E))

            zt = st_sb_pool.tile([1, 1], F32)
            nc.vector.tensor_reduce(zt[:], szv[:], axis=mybir.AxisListType.X, op=OP.add)
            nc.vector.tensor_scalar_mul(zt[:], zt[:], 0.001 / T)

            el = st_sb_pool.tile([1, 1], F32)
            nc.vector.tensor_reduce(el[:], slv[:], axis=mybir.AxisListType.X, op=OP.add)
            # entropy = -sum/T ; el = (ln E - entropy) * 0.01 = sum*(0.01/T) + 0.01*lnE
            nc.vector.tensor_scalar(
                out=el[:], in0=el[:], scalar1=0.01 / T, scalar2=float(0.01 * np.log(E)),
                op0=OP.mult, op1=OP.add,
            )

            ug = st_sb_pool.tile([1, E], F32)
            nc.vector.tensor_scalar(out=ug[:], in0=tpe[:], scalar1=0.01, scalar2=None, op0=OP.is_gt)
            ul = st_sb_pool.tile([1, 1], F32)
            nc.vector.tensor_reduce(ul[:], ug[:], axis=mybir.AxisListType.X, op=OP.add)
            # util = (1 - usage/E) * 0.1 = -us*(0.1/E) + 0.1
            nc.vector.tensor_scalar(
                out=ul[:], in0=ul[:], scalar1=-0.1 / E, scalar2=0.1, op0=OP.mult, op1=OP.add,
            )

            auxv = st_sb_pool.tile([1, 1], F32)
            nc.vector.tensor_tensor(auxv[:], lb[:], zt[:], OP.add)
            nc.vector.tensor_tensor(auxv[:], auxv[:], el[:], OP.add)
            nc.vector.tensor_tensor(auxv[:], auxv[:], ul[:], OP.add)
            nc.vector.tensor_scalar(
                out=auxv[:], in0=auxv[:], scalar1=100.0, scalar2=0.0, op0=OP.min, op1=OP.max,
            )
            nc.sync.dma_start(out=aux_ext, in_=auxv[:])

        nc.gpsimd.dma_start(out=wd_sb[:], in_=wd_ext.rearrange("(k p) h -> p k h", p=128))
        nc.gpsimd.dma_start(out=swg_sb[:], in_=swg_ext.rearrange("(k p) i -> p k i", p=128))
        nc.gpsimd.dma_start(out=swu_sb[:], in_=swu_ext.rearrange("(k p) i -> p k i", p=128))
        nc.gpsimd.dma_start(out=swd_sb[:], in_=swd_ext.rearrange("(k p) h -> p k h", p=128))

        # ---------------- Phase B: routed expert FFN (dense, masked) -----
        # ---------------- Phase C: shared expert slice --------------------
        with (
            tc.tile_pool(name="wstream", bufs=3) as wstream,
            tc.tile_pool(name="gact", bufs=2) as gact_pool,
            tc.tile_pool(name="hT", bufs=1) as hT_pool,
            tc.tile_pool(name="osb", bufs=3) as osb_pool,
            tc.tile_pool(name="gps", bufs=1, space="PSUM") as g_pool,
            tc.tile_pool(name="ups", bufs=1, space="PSUM") as u_pool,
            tc.tile_pool(name="ops", bufs=2, space="PSUM") as o_pool,
        ):
            hT = hT_pool.tile([128, KI, TC], BF16, tag="hT")
            hsT = hT_pool.tile([128, ISL // 128, TC], BF16, tag="hsT")

            def _silu_mul(dst, g_ps, u_ps):
                ga = gact_pool.tile([128, TC], BF16, tag="ga")
                if native_silu:
                    nc.scalar.activation(ga[:], g_ps[:], AF.Silu)
                else:
                    sgm = gact_pool.tile([128, TC], F32, tag="sgm")
                    nc.scalar.activation(sgm[:], g_ps[:], AF.Sigmoid)
                    nc.vector.tensor_tensor(ga[:], sgm[:], g_ps[:], OP.mult)
                nc.vector.tensor_tensor(dst, ga[:], u_ps[:], OP.mult)
            for ch in range(NCH):
                t0 = ch * TC
                # gate/up for the routed expert
                for i in range(KI):
                    wg_sb = wstream.tile([128, KH, 128], BF16, tag="wg")
                    wu_sb = wstream.tile([128, KH, 128], BF16, tag="wu")
                    if ch == 0:
                        # first pass: cast-load fp32 -> bf16 (SWDGE), then cache
                        # the bf16 tiles in DRAM so later chunks reload via
                        # HWDGE and keep the gpsimd queue free for collectives
                        nc.gpsimd.dma_start(
                            out=wg_sb[:],
                            in_=AP(wg_ext.tensor, i * 128, [[I_DIM, 128], [128 * I_DIM, KH], [1, 128]]),
                        )
                        nc.gpsimd.dma_start(
                            out=wu_sb[:],
                            in_=AP(wu_ext.tensor, i * 128, [[I_DIM, 128], [128 * I_DIM, KH], [1, 128]]),
                        )
                        if NCH > 1:
                            nc.scalar.dma_start(out=wgc[i], in_=wg_sb[:])
                            nc.scalar.dma_start(out=wuc[i], in_=wu_sb[:])
                    else:
                        nc.sync.dma_start(out=wg_sb[:].rearrange("p a b -> p (a b)"), in_=wgc[i])
                        nc.sync.dma_start(out=wu_sb[:].rearrange("p a b -> p (a b)"), in_=wuc[i])
                    g_ps = g_pool.tile([128, TC], F32, tag="g")
                    u_ps = u_pool.tile([128, TC], F32, tag="u")
                    for half in range(TC // 512):
                        hs = slice(half * 512, (half + 1) * 512)
                        for k in range(KH):
                            nc.tensor.matmul(
                                g_ps[:, hs], wg_sb[:, k, :],
                                xT_bf[:, k, t0 + half * 512: t0 + (half + 1) * 512],
                                start=(k == 0), stop=(k == KH - 1),
                            )
                        for k in range(KH):
                            nc.tensor.matmul(
                                u_ps[:, hs], wu_sb[:, k, :],
                                xT_bf[:, k, t0 + half * 512: t0 + (half + 1) * 512],
                                start=(k == 0), stop=(k == KH - 1),
                            )
                    _silu_mul(hT[:, i, :], g_ps, u_ps)
                # gate/up for the shared expert slice
                for i2 in range(ISL // 128):
                    g_ps = g_pool.tile([128, TC], F32, tag="g")
                    u_ps = u_pool.tile([128, TC], F32, tag="u")
                    for half in range(TC // 512):
                        hs = slice(half * 512, (half + 1) * 512)
                        for k in range(KH):
                            nc.tensor.matmul(
                                g_ps[:, hs], swg_sb[:, k, i2 * 128:(i2 + 1) * 128],
                                xT_bf[:, k, t0 + half * 512: t0 + (half + 1) * 512],
                                start=(k == 0), stop=(k == KH - 1),
                            )
                        for k in range(KH):
                            nc.tensor.matmul(
                                u_ps[:, hs], swu_sb[:, k, i2 * 128:(i2 + 1) * 128],
                                xT_bf[:, k, t0 + half * 512: t0 + (half + 1) * 512],
                                start=(k == 0), stop=(k == KH - 1),
                            )
                    _silu_mul(hsT[:, i2, :], g_ps, u_ps)
                # down projections
                for ts_ in range(NTS):
                    jj = ch * NTS + ts_
                    tsl = slice(ts_ * 128, (ts_ + 1) * 128)
                    o_ps = o_pool.tile([128, H], F32, tag="o")
                    for half in range(H // 512):
                        hs = slice(half * 512, (half + 1) * 512)
                        for i in range(KI):
                            nc.tensor.matmul(
                                o_ps[:, hs], hT[:, i, tsl], wd_sb[:, i, hs],
                                start=(i == 0), stop=(i == KI - 1),
                            )
                    o_sb = osb_pool.tile([128, H], F32, tag="osb")
                    nc.vector.tensor_scalar_mul(o_sb[:], o_ps[:], cwc_all[:, jj:jj + 1])
                    nc.scalar.dma_start(out=out_accs[ch][ts_ * 128:(ts_ + 1) * 128, :], in_=o_sb[:])
                    # shared down, accumulated into out_acc by DMA
                    o2_ps = o_pool.tile([128, H], F32, tag="o")
                    for half in range(H // 512):
                        hs = slice(half * 512, (half + 1) * 512)
                        for i2 in range(ISL // 128):
                            nc.tensor.matmul(
                                o2_ps[:, hs], hsT[:, i2, tsl], swd_sb[:, i2, hs],
                                start=(i2 == 0), stop=(i2 == ISL // 128 - 1),
                            )
                    o2_sb = osb_pool.tile([128, H], F32, tag="osb")
                    nc.vector.tensor_scalar_mul(o2_sb[:], o2_ps[:], sig_sb[:])
                    nc.gpsimd.dma_start(
                        out=out_accs[ch][ts_ * 128:(ts_ + 1) * 128, :], in_=o2_sb[:],
                        accum_op=OP.add,
                    )
                # this chunk of tokens is complete: reduce-scatter it now so the
                # collective overlaps the next chunk's compute
                nc.gpsimd.collective_compute(
                    "ReduceScatter",
                    OP.add,
                    ins=[out_accs[ch].opt()],
                    outs=[rs_outs[ch].opt()],
                    replica_groups=[list(range(N_CORES))],
                )
                nc.scalar.dma_start(
                    out=out_ext[ch * RSC:(ch + 1) * RSC, :], in_=rs_outs[ch][:]
                )


    nc.compile()
    return nc


_CACHED = {}


def _get_program(T):
    if T not in _CACHED:
        _CACHED[T] = build_program(T)
    return _CACHED[T]


def make_in_maps(inputs: dict, T: int):
    x = np.ascontiguousarray(np.asarray(inputs["hidden_states"], dtype=np.float32).reshape(T, H))
    rwT = np.ascontiguousarray(np.asarray(inputs["router_w"], dtype=np.float32).T)
    wg = np.asarray(inputs["wg"], dtype=np.float32)
    wu = np.asarray(inputs["wu"], dtype=np.float32)
    wd = np.asarray(inputs["wd"], dtype=np.float32)
    swg = np.asarray(inputs["shared_wg"], dtype=np.float32)
    swu = np.asarray(inputs["shared_wu"], dtype=np.float32)
    swd = np.asarray(inputs["shared_wd"], dtype=np.float32)
    gate = float(np.asarray(inputs["shared_gate"]).reshape(-1)[0])
    in_maps = []
    for c in range(N_CORES):
        sel = np.zeros((128, E), dtype=np.float32)
        sel[:, c] = 1.0
        in_maps.append({
            "x": x,
            "rwT": rwT,
            "wg": np.ascontiguousarray(wg[c]),
            "wu": np.ascontiguousarray(wu[c]),
            "wd": np.ascontiguousarray(wd[c]),
            "swg": np.ascontiguousarray(swg[:, c * ISL:(c + 1) * ISL]),
            "swu": np.ascontiguousarray(swu[:, c * ISL:(c + 1) * ISL]),
            "swd": np.ascontiguousarray(swd[c * ISL:(c + 1) * ISL, :]),
            "sel": sel,
            "sgate": np.full((128, 1), gate, dtype=np.float32),
        })
    return in_maps


def assemble_out(results, T):
    # core c's "out_rs" rows are [chunk, 128] with global token
    # t = chunk*TC + c*128 + r (chunked ReduceScatter layout)
    TC = min(1024, T)
    NCH = T // TC
    stack = np.stack([results[c]["out_rs"] for c in range(N_CORES)], axis=0)
    stack = stack.reshape(N_CORES, NCH, TC // N_CORES, H).transpose(1, 0, 2, 3)
    return stack.reshape(T, H)


def kernel(**inputs):
    hs = np.asarray(inputs["hidden_states"])
    B, S, _ = hs.shape
    T = B * S
    nc = _get_program(T)
    in_maps = make_in_maps(inputs, T)
    res = run_bass_kernel_spmd(nc, in_maps, list(range(N_CORES)))
    out = assemble_out(res.results, T)
    aux = np.float32(res.results[0]["aux"].reshape(())[()])
    return out.reshape(B, S, H), aux


# revision 23
# speedup vs baseline: 1.1204x; 1.0255x over previous
"""Trainium2 Bass kernel for an 8-expert top-2 MoE layer with shared expert.

Sharding: expert-parallel. Each of the 8 cores owns one expert's FFN weights
plus a 1/8 slice (intermediate dim) of the shared expert. hidden_states and
the router are replicated; each core computes the router for all tokens in
fp32 (top-2 selection is precision-critical), the dense-masked FFN for its
own expert in bf16 (fp32 accumulation), and its shared-expert slice. Partial
outputs are summed with an on-device ReduceScatter; the host concatenates
the 8 token-slices. aux losses are computed redundantly on every core.

Self-contained: shapes hardcoded for B=2, S=2048, H=1024, I=2048, E=8.
"""

import sys

sys.path.insert(0, "/opt/trn_rl_repo")

from contextlib import ExitStack

import numpy as np

import concourse.bacc as bacc
import concourse.mybir as mybir
from concourse import masks, tile
from concourse.bass_types import AP
from concourse.bass_utils import run_bass_kernel_spmd

F32 = mybir.dt.float32
BF16 = mybir.dt.bfloat16
AF = mybir.ActivationFunctionType
OP = mybir.AluOpType

N_CORES = 8
H = 1024
I_DIM = 2048
E = 8
ISL = I_DIM // N_CORES  # shared-expert intermediate slice per core
KH = H // 128           # 8 k-tiles over H
KI = I_DIM // 128       # 16 k-tiles over I
EPS = 1e-6


def _bc_last(ap: AP, n: int) -> AP:
    """Broadcast an AP along a new innermost dim of size n (stride 0)."""
    return AP(ap.tensor, ap.offset, [list(x) for x in ap.ap] + [[0, n]])


def _bc_mid(ap: AP, n: int) -> AP:
    """[128, F] -> [128, n, F] broadcast on the middle dim (stride 0)."""
    a = [list(x) for x in ap.ap]
    return AP(ap.tensor, ap.offset, [a[0], [0, n], *a[1:]])


def build_program(T: int = 4096, tc_tokens: int = 1024, native_silu: bool = True):
    assert T % 1024 == 0 or T in (512,), T
    TC = min(tc_tokens, T)
    NT = T // 128          # token tiles
    NCH = T // TC          # ffn token chunks
    NTS = TC // 128        # token tiles per chunk
    TSL = T // N_CORES     # output slice per core after ReduceScatter

    nc = bacc.Bacc(
        "TRN2",
        target_bir_lowering=False,
        debug=False,
        enable_asserts=True,
        num_devices=N_CORES,
    )

    x_ext = nc.dram_tensor("x", [T, H], F32, kind="ExternalInput").ap()
    rwT_ext = nc.dram_tensor("rwT", [H, E], F32, kind="ExternalInput").ap()
    wg_ext = nc.dram_tensor("wg", [H, I_DIM], F32, kind="ExternalInput").ap()
    wu_ext = nc.dram_tensor("wu", [H, I_DIM], F32, kind="ExternalInput").ap()
    wd_ext = nc.dram_tensor("wd", [I_DIM, H], F32, kind="ExternalInput").ap()
    swg_ext = nc.dram_tensor("swg", [H, ISL], F32, kind="ExternalInput").ap()
    swu_ext = nc.dram_tensor("swu", [H, ISL], F32, kind="ExternalInput").ap()
    swd_ext = nc.dram_tensor("swd", [ISL, H], F32, kind="ExternalInput").ap()
    sel_ext = nc.dram_tensor("sel", [128, E], F32, kind="ExternalInput").ap()
    sgate_ext = nc.dram_tensor("sgate", [128, 1], F32, kind="ExternalInput").ap()

    out_ext = nc.dram_tensor("out_rs", [TSL, H], F32, kind="ExternalOutput").ap()
    aux_ext = nc.dram_tensor("aux", [1, 1], F32, kind="ExternalOutput").ap()

    with tile.TileContext(nc) as tc, ExitStack() as top:
        dram = top.enter_context(tc.tile_pool(name="dram", bufs=1, space="DRAM"))
        RSC = TC // N_CORES                   # rows per core per chunk
        out_accs = [dram.tile([TC, H], F32, name=f"acc{q}", tag=f"acc{q}") for q in range(NCH)]
        rs_outs = [dram.tile([RSC, H], F32, name=f"rsq{q}", tag=f"rsq{q}") for q in range(NCH)]
        wgc = dram.tile([KI, 128, KH * 128], BF16, name="wgc", tag="wgc")
        wuc = dram.tile([KI, 128, KH * 128], BF16, name="wuc", tag="wuc")
        warm_in = dram.tile([N_CORES, 64], F32, name="warm_in", tag="warm_in")
        warm_out = dram.tile([1, 64], F32, name="warm_out", tag="warm_out")

        const_pool = top.enter_context(tc.tile_pool(name="const", bufs=1))
        ident = const_pool.tile([128, 128], F32)
        masks.make_identity(nc, ident[:])
        ones = const_pool.tile([128, 1], F32)
        nc.vector.memset(ones[:], 1.0)
        rwT_sb = const_pool.tile([128, KH, E], F32)
        nc.sync.dma_start(out=rwT_sb[:], in_=rwT_ext.rearrange("(k p) e -> p k e", p=128))
        sel_sb = const_pool.tile([128, E], F32)
        nc.sync.dma_start(out=sel_sb[:], in_=sel_ext)
        sig_sb = const_pool.tile([128, 1], F32)
        sg_in = const_pool.tile([128, 1], F32)
        nc.sync.dma_start(out=sg_in[:], in_=sgate_ext)
        nc.scalar.activation(sig_sb[:], sg_in[:], AF.Sigmoid)

        # Resident activations / weights
        big_pool = top.enter_context(tc.tile_pool(name="resident", bufs=1))
        xT_bf = big_pool.tile([128, KH, T], BF16)          # x^T, bf16
        wd_sb = big_pool.tile([128, KI, H], BF16)          # wd tiles [i_k][i_p, h]
        swg_sb = big_pool.tile([128, KH, ISL], BF16)
        swu_sb = big_pool.tile([128, KH, ISL], BF16)
        swd_sb = big_pool.tile([128, ISL // 128, H], BF16)

        # Router tensors (fp32), persistent until aux is finalized
        rt_pool = top.enter_context(tc.tile_pool(name="router", bufs=1))
        logits_all = rt_pool.tile([128, NT, E], F32)
        probs = rt_pool.tile([128, NT, E], F32)
        mask2 = rt_pool.tile([128, NT, E], F32)
        pl = rt_pool.tile([128, NT, E], F32)
        zsq = rt_pool.tile([128, NT], F32)
        cwc_all = rt_pool.tile([128, NT], F32)

        # ---------------- Phase A: x load + transpose + router logits ----
        with (
            tc.tile_pool(name="xin", bufs=3) as xin_pool,
            tc.tile_pool(name="xtT", bufs=2) as xtT_pool,
            tc.tile_pool(name="trps", bufs=2, space="PSUM") as tr_pool,
            tc.tile_pool(name="lgps", bufs=2, space="PSUM") as lg_pool,
        ):
            for j in range(NT):
                xt = xin_pool.tile([128, H], F32)
                nc.sync.dma_start(out=xt[:], in_=x_ext[j * 128:(j + 1) * 128, :])
                tr_ps = tr_pool.tile([128, H], F32)
                for k in range(KH):
                    nc.tensor.transpose(
                        tr_ps[:, k * 128:(k + 1) * 128],
                        xt[:, k * 128:(k + 1) * 128],
                        ident[:],
                    )
                xtT = xtT_pool.tile([128, H], F32)
                nc.scalar.activation(xtT[:], tr_ps[:], AF.Copy)
                nc.vector.tensor_copy(
                    xT_bf[:, :, j * 128:(j + 1) * 128],
                    xtT[:].rearrange("p (k t) -> p k t", k=KH),
                )
                lg_ps = lg_pool.tile([128, E], F32)
                for k in range(KH):
                    nc.tensor.matmul(
                        lg_ps[:],
                        xtT[:, k * 128:(k + 1) * 128],
                        rwT_sb[:, k, :],
                        start=(k == 0),
                        stop=(k == KH - 1),
                    )
                nc.vector.tensor_scalar(
                    out=logits_all[:, j, :], in0=lg_ps[:],
                    scalar1=50.0, scalar2=-50.0, op0=OP.min, op1=OP.max,
                )

        wz = const_pool.tile([N_CORES, 64], F32)
        nc.vector.memset(wz[:], 0.0)
        nc.sync.dma_start(out=warm_in[:], in_=wz[:])
        nc.gpsimd.collective_compute(
            "ReduceScatter",
            OP.add,
            ins=[warm_in.opt()],
            outs=[warm_out.opt()],
            replica_groups=[list(range(N_CORES))],
        )

        # ---------------- Router math (batched over all tokens) ----------
        with tc.tile_pool(name="rtmp", bufs=1) as rt:
            mx = rt.tile([128, NT], F32)
            nc.vector.tensor_reduce(mx[:], logits_all[:], axis=mybir.AxisListType.X, op=OP.max)
            shifted = rt.tile([128, NT, E], F32)
            nc.vector.tensor_tensor(shifted[:], logits_all[:], _bc_last(mx[:], E), OP.subtract)
            exps = rt.tile([128, NT, E], F32)
            nc.scalar.activation(exps[:], shifted[:], AF.Exp)
            sums = rt.tile([128, NT], F32)
            nc.vector.tensor_reduce(sums[:], exps[:], axis=mybir.AxisListType.X, op=OP.add)
            rcp = rt.tile([128, NT], F32)
            nc.vector.reciprocal(rcp[:], sums[:])
            nc.vector.tensor_tensor(probs[:], exps[:], _bc_last(rcp[:], E), OP.mult)
            # z-loss: lse = ln(sum) + max, squared
            lns = rt.tile([128, NT], F32)
            nc.scalar.activation(lns[:], sums[:], AF.Ln)
            lse = rt.tile([128, NT], F32)
            nc.vector.tensor_tensor(lse[:], lns[:], mx[:], OP.add)
            nc.vector.tensor_tensor(zsq[:], lse[:], lse[:], OP.mult)
            # top-2 selection on LOGITS (same order as probs; avoids any
            # dependence of the selection on exp-LUT rounding)
            m1 = rt.tile([128, NT], F32)
            nc.vector.tensor_reduce(m1[:], logits_all[:], axis=mybir.AxisListType.X, op=OP.max)
            eq1 = rt.tile([128, NT, E], F32)
            nc.vector.tensor_tensor(eq1[:], logits_all[:], _bc_last(m1[:], E), OP.is_ge)
            pm = rt.tile([128, NT, E], F32)
            nc.vector.tensor_scalar_mul(pm[:], eq1[:], 1.0e9)
            nc.vector.tensor_tensor(pm[:], logits_all[:], pm[:], OP.subtract)
            m2 = rt.tile([128, NT], F32)
            nc.vector.tensor_reduce(m2[:], pm[:], axis=mybir.AxisListType.X, op=OP.max)
            nc.vector.tensor_tensor(mask2[:], logits_all[:], _bc_last(m2[:], E), OP.is_ge)
            # cw = probs * mask2 / (sum of selected probs + EPS)
            cw = rt.tile([128, NT, E], F32)
            nc.vector.tensor_tensor(cw[:], probs[:], mask2[:], OP.mult)
            den = rt.tile([128, NT], F32)
            nc.vector.tensor_reduce(den[:], cw[:], axis=mybir.AxisListType.X, op=OP.add)
            nc.vector.tensor_scalar_add(den[:], den[:], EPS)
            rd = rt.tile([128, NT], F32)
            nc.vector.reciprocal(rd[:], den[:])
            nc.vector.tensor_tensor(cw[:], cw[:], _bc_last(rd[:], E), OP.mult)
            cwsel = rt.tile([128, NT, E], F32)
            nc.vector.tensor_tensor(cwsel[:], cw[:], _bc_mid(sel_sb[:], NT), OP.mult)
            nc.vector.tensor_reduce(cwc_all[:], cwsel[:], axis=mybir.AxisListType.X, op=OP.add)
            # entropy terms
            psafe = rt.tile([128, NT, E], F32)
            nc.vector.tensor_scalar_max(psafe[:], probs[:], EPS)
            lp = rt.tile([128, NT, E], F32)
            nc.scalar.activation(lp[:], psafe[:], AF.Ln)
            nc.vector.tensor_tensor(pl[:], psafe[:], lp[:], OP.mult)

        # ---------------- aux loss reductions ----------------------------
        with (
            tc.tile_pool(name="stps", bufs=1, space="PSUM") as st_pool,
            tc.tile_pool(name="stsb", bufs=1) as st_sb_pool,
        ):
            stm = st_pool.tile([1, NT * E], F32)
            stp = st_pool.tile([1, NT * E], F32)
            stz = st_pool.tile([1, NT], F32)
            stl = st_pool.tile([1, NT * E], F32)
            nc.tensor.matmul(stm[:], ones[:], mask2[:].rearrange("p a b -> p (a b)"), start=True, stop=True)
            nc.tensor.matmul(stp[:], ones[:], probs[:].rearrange("p a b -> p (a b)"), start=True, stop=True)
            nc.tensor.matmul(stz[:], ones[:], zsq[:], start=True, stop=True)
            nc.tensor.matmul(stl[:], ones[:], pl[:].rearrange("p a b -> p (a b)"), start=True, stop=True)

            smv = st_sb_pool.tile([1, NT * E], F32)
            spv = st_sb_pool.tile([1, NT * E], F32)
            szv = st_sb_pool.tile([1, NT], F32)
            slv = st_sb_pool.tile([1, NT * E], F32)
            nc.scalar.activation(smv[:], stm[:], AF.Copy)
            nc.scalar.activation(spv[:], stp[:], AF.Copy)
            nc.scalar.activation(szv[:], stz[:], AF.Copy)
            nc.scalar.activation(slv[:], stl[:], AF.Copy)

            def _sum_over_tiles(dst, src):
                # src [1, NT*E] viewed as [1, E, NT] (strided) -> reduce X
                v = AP(src.tensor, src.offset, [list(src.ap[0]), [1, E], [E, NT]])
                nc.vector.tensor_reduce(dst, v, axis=mybir.AxisListType.X, op=OP.add)

            tpe = st_sb_pool.tile([1, E], F32)
            avg = st_sb_pool.tile([1, E], F32)
            _sum_over_tiles(tpe[:], smv[:])
            _sum_over_tiles(avg[:], spv[:])
            nc.vector.tensor_scalar_mul(tpe[:], tpe[:], 1.0 / (2 * T))
            nc.vector.tensor_scalar_mul(avg[:], avg[:], 1.0 / T)
            prod = st_sb_pool.tile([1, E], F32)
            nc.vector.tensor_tensor(prod[:], tpe[:], avg[:], OP.mult)
            lb = st_sb_pool.tile([1, 1], F32)
            nc.vector.tensor_reduce(lb[:], prod[:], axis=mybir.AxisListType.X, op=OP.add)
            nc.vector.tensor_scalar_mul(lb[:], lb[:], float(E))

            zt = st_sb_pool.tile([1, 1], F32)
            nc.vector.tensor_reduce(zt[:], szv[:], axis=mybir.AxisListType.X, op=OP.add)
            nc.vector.tensor_scalar_mul(zt[:], zt[:], 0.001 / T)

            el = st_sb_pool.tile([1, 1], F32)
            nc.vector.tensor_reduce(el[:], slv[:], axis=mybir.AxisListType.X, op=OP.add)
            # entropy = -sum/T ; el = (ln E - entropy) * 0.01 = sum*(0.01/T) + 0.01*lnE
            nc.vector.tensor_scalar(
                out=el[:], in0=el[:], scalar1=0.01 / T, scalar2=float(0.01 * np.log(E)),
                op0=OP.mult, op1=OP.add,
            )

            ug = st_sb_pool.tile([1, E], F32)
            nc.vector.tensor_scalar(out=ug[:], in0=tpe[:], scalar1=0.01, scalar2=None, op0=OP.is_gt)
            ul = st_sb_pool.tile([1, 1], F32)
            nc.vector.tensor_reduce(ul[:], ug[:], axis=mybir.AxisListType.X, op=OP.add)
            # util = (1 - usage/E) * 0.1 = -us*(0.1/E) + 0.1
            nc.vector.tensor_scalar(
                out=ul[:], in0=ul[:], scalar1=-0.1 / E, scalar2=0.1, op0=OP.mult, op1=OP.add,
            )

            auxv = st_sb_pool.tile([1, 1], F32)
            nc.vector.tensor_tensor(auxv[:], lb[:], zt[:], OP.add)
            nc.vector.tensor_tensor(auxv[:], auxv[:], el[:], OP.add)
            nc.vector.tensor_tensor(auxv[:], auxv[:], ul[:], OP.add)
            nc.vector.tensor_scalar(
                out=auxv[:], in0=auxv[:], scalar1=100.0, scalar2=0.0, op0=OP.min, op1=OP.max,
            )
            nc.sync.dma_start(out=aux_ext, in_=auxv[:])

        nc.gpsimd.dma_start(out=wd_sb[:], in_=wd_ext.rearrange("(k p) h -> p k h", p=128))
        nc.gpsimd.dma_start(out=swg_sb[:], in_=swg_ext.rearrange("(k p) i -> p k i", p=128))
        nc.gpsimd.dma_start(out=swu_sb[:], in_=swu_ext.rearrange("(k p) i -> p k i", p=128))
        nc.gpsimd.dma_start(out=swd_sb[:], in_=swd_ext.rearrange("(k p) h -> p k h", p=128))

        # ---------------- Phase B: routed expert FFN (dense, masked) -----
        # ---------------- Phase C: shared expert slice --------------------
        with (
            tc.tile_pool(name="wstream", bufs=3) as wstream,
            tc.tile_pool(name="gact", bufs=2) as gact_pool,
            tc.tile_pool(name="hT", bufs=1) as hT_pool,
            tc.tile_pool(name="osb", bufs=3) as osb_pool,
            tc.tile_pool(name="gps", bufs=1, space="PSUM") as g_pool,
            tc.tile_pool(name="ups", bufs=1, space="PSUM") as u_pool,
            tc.tile_pool(name="ops", bufs=2, space="PSUM") as o_pool,
        ):
            hT = hT_pool.tile([128, KI, TC], BF16, tag="hT")
            hsT = hT_pool.tile([128, ISL // 128, TC], BF16, tag="hsT")

            def _silu_mul(dst, g_ps, u_ps):
                ga = gact_pool.tile([128, TC], BF16, tag="ga")
                if native_silu:
                    nc.scalar.activation(ga[:], g_ps[:], AF.Silu)
                else:
                    sgm = gact_pool.tile([128, TC], F32, tag="sgm")
                    nc.scalar.activation(sgm[:], g_ps[:], AF.Sigmoid)
                    nc.vector.tensor_tensor(ga[:], sgm[:], g_ps[:], OP.mult)
                nc.vector.tensor_tensor(dst, ga[:], u_ps[:], OP.mult)
            for ch in range(NCH):
                t0 = ch * TC
                # gate/up for the routed expert
                for i in range(KI):
                    wg_sb = wstream.tile([128, KH, 128], BF16, tag="wg")
                    wu_sb = wstream.tile([128, KH, 128], BF16, tag="wu")
                    if ch == 0:
                        # first pass: cast-load fp32 -> bf16 (SWDGE), then cache
                        # the bf16 tiles in DRAM so later chunks reload via
                        # HWDGE and keep the gpsimd queue free for collectives
                        nc.gpsimd.dma_start(
                            out=wg_sb[:],
                            in_=AP(wg_ext.tensor, i * 128, [[I_DIM, 128], [128 * I_DIM, KH], [1, 128]]),
                        )
                        nc.gpsimd.dma_start(
                            out=wu_sb[:],
                            in_=AP(wu_ext.tensor, i * 128, [[I_DIM, 128], [128 * I_DIM, KH], [1, 128]]),
                        )
                        if NCH > 1:
                            nc.scalar.dma_start(out=wgc[i], in_=wg_sb[:])
                            nc.scalar.dma_start(out=wuc[i], in_=wu_sb[:])
                    else:
                        nc.sync.dma_start(out=wg_sb[:].rearrange("p a b -> p (a b)"), in_=wgc[i])
                        nc.sync.dma_start(out=wu_sb[:].rearrange("p a b -> p (a b)"), in_=wuc[i])
                    g_ps = g_pool.tile([128, TC], F32, tag="g")
                    u_ps = u_pool.tile([128, TC], F32, tag="u")
                    for half in range(TC // 512):
                        hs = slice(half * 512, (half + 1) * 512)
                        for k in range(KH):
                            nc.tensor.matmul(
                                g_ps[:, hs], wg_sb[:, k, :],
                                xT_bf[:, k, t0 + half * 512: t0 + (half + 1) * 512],
                                start=(k == 0), stop=(k == KH - 1),
                            )
                        for k in range(KH):
                            nc.tensor.matmul(
                                u_ps[:, hs], wu_sb[:, k, :],
                                xT_bf[:, k, t0 + half * 512: t0 + (half + 1) * 512],
                                start=(k == 0), stop=(k == KH - 1),
                            )
                    _silu_mul(hT[:, i, :], g_ps, u_ps)
                # gate/up for the shared expert slice
                for i2 in range(ISL // 128):
                    g_ps = g_pool.tile([128, TC], F32, tag="g")
                    u_ps = u_pool.tile([128, TC], F32, tag="u")
                    for half in range(TC // 512):
                        hs = slice(half * 512, (half + 1) * 512)
                        for k in range(KH):
                            nc.tensor.matmul(
                                g_ps[:, hs], swg_sb[:, k, i2 * 128:(i2 + 1) * 128],
                                xT_bf[:, k, t0 + half * 512: t0 + (half + 1) * 512],
                                start=(k == 0), stop=(k == KH - 1),
                            )
                        for k in range(KH):
                            nc.tensor.matmul(
                                u_ps[:, hs], swu_sb[:, k, i2 * 128:(i2 + 1) * 128],
                                xT_bf[:, k, t0 + half * 512: t0 + (half + 1) * 512],
                                start=(k == 0), stop=(k == KH - 1),
                            )
                    _silu_mul(hsT[:, i2, :], g_ps, u_ps)
                # down projections
                for ts_ in range(NTS):
                    jj = ch * NTS + ts_
                    tsl = slice(ts_ * 128, (ts_ + 1) * 128)
                    o_ps = o_pool.tile([128, H], F32, tag="o")
                    for half in range(H // 512):
                        hs = slice(half * 512, (half + 1) * 512)
                        for i in range(KI):
                            nc.tensor.matmul(
                                o_ps[:, hs], hT[:, i, tsl], wd_sb[:, i, hs],
                                start=(i == 0), stop=(i == KI - 1),
                            )
                    o_sb = osb_pool.tile([128, H], F32, tag="osb")
                    nc.vector.tensor_scalar_mul(o_sb[:], o_ps[:], cwc_all[:, jj:jj + 1])
                    nc.scalar.dma_start(out=out_accs[ch][ts_ * 128:(ts_ + 1) * 128, :], in_=o_sb[:])
                    # shared down, accumulated into out_acc by DMA
                    o2_ps = o_pool.tile([128, H], F32, tag="o")
                    for half in range(H // 512):
                        hs = slice(half * 512, (half + 1) * 512)
                        for i2 in range(ISL // 128):
                            nc.tensor.matmul(
                                o2_ps[:, hs], hsT[:, i2, tsl], swd_sb[:, i2, hs],
                                start=(i2 == 0), stop=(i2 == ISL // 128 - 1),
                            )
                    o2_sb = osb_pool.tile([128, H], F32, tag="osb")
                    nc.vector.tensor_scalar_mul(o2_sb[:], o2_ps[:], sig_sb[:])
                    nc.gpsimd.dma_start(
                        out=out_accs[ch][ts_ * 128:(ts_ + 1) * 128, :], in_=o2_sb[:],
                        accum_op=OP.add,
                    )
                # this chunk of tokens is complete: reduce-scatter it now so the
                # collective overlaps the next chunk's compute
                nc.gpsimd.collective_compute(
                    "ReduceScatter",
                    OP.add,
                    ins=[out_accs[ch].opt()],
                    outs=[rs_outs[ch].opt()],
                    replica_groups=[list(range(N_CORES))],
                )
                nc.scalar.dma_start(
                    out=out_ext[ch * RSC:(ch + 1) * RSC, :], in_=rs_outs[ch][:]
                )


    nc.compile()
    return nc


_CACHED = {}


def _get_program(T):
    if T not in _CACHED:
        _CACHED[T] = build_program(T)
    return _CACHED[T]


def make_in_maps(inputs: dict, T: int):
    x = np.ascontiguousarray(np.asarray(inputs["hidden_states"], dtype=np.float32).reshape(T, H))
    rwT = np.ascontiguousarray(np.asarray(inputs["router_w"], dtype=np.float32).T)
    wg = np.asarray(inputs["wg"], dtype=np.float32)
    wu = np.asarray(inputs["wu"], dtype=np.float32)
    wd = np.asarray(inputs["wd"], dtype=np.float32)
    swg = np.asarray(inputs["shared_wg"], dtype=np.float32)
    swu = np.asarray(inputs["shared_wu"], dtype=np.float32)
    swd = np.asarray(inputs["shared_wd"], dtype=np.float32)
    gate = float(np.asarray(inputs["shared_gate"]).reshape(-1)[0])
    in_maps = []
    for c in range(N_CORES):
        sel = np.zeros((128, E), dtype=np.float32)
        sel[:, c] = 1.0
        in_maps.append({
            "x": x,
            "rwT": rwT,
            "wg": np.ascontiguousarray(wg[c]),
            "wu": np.ascontiguousarray(wu[c]),
            "wd": np.ascontiguousarray(wd[c]),
            "swg": np.ascontiguousarray(swg[:, c * ISL:(c + 1) * ISL]),
            "swu": np.ascontiguousarray(swu[:, c * ISL:(c + 1) * ISL]),
            "swd": np.ascontiguousarray(swd[c * ISL:(c + 1) * ISL, :]),
            "sel": sel,
            "sgate": np.full((128, 1), gate, dtype=np.float32),
        })
    return in_maps


def assemble_out(results, T):
    # core c's "out_rs" rows are [chunk, 128] with global token
    # t = chunk*TC + c*128 + r (chunked ReduceScatter layout)
    TC = min(1024, T)
    NCH = T // TC
    stack = np.stack([results[c]["out_rs"] for c in range(N_CORES)], axis=0)
    stack = stack.reshape(N_CORES, NCH, TC // N_CORES, H).transpose(1, 0, 2, 3)
    return stack.reshape(T, H)


def kernel(**inputs):
    hs = np.asarray(inputs["hidden_states"])
    B, S, _ = hs.shape
    T = B * S
    nc = _get_program(T)
    in_maps = make_in_maps(inputs, T)
    res = run_bass_kernel_spmd(nc, in_maps, list(range(N_CORES)))
    out = assemble_out(res.results, T)
    aux = np.float32(res.results[0]["aux"].reshape(())[()])
    return out.reshape(B, S, H), aux


# revision 24
# speedup vs baseline: 1.1221x; 1.0015x over previous
"""Trainium2 Bass kernel for an 8-expert top-2 MoE layer with shared expert.

Sharding: expert-parallel. Each of the 8 cores owns one expert's FFN weights
plus a 1/8 slice (intermediate dim) of the shared expert. hidden_states and
the router are replicated; each core computes the router for all tokens in
fp32 (top-2 selection is precision-critical), the dense-masked FFN for its
own expert in bf16 (fp32 accumulation), and its shared-expert slice. Partial
outputs are summed with an on-device ReduceScatter; the host concatenates
the 8 token-slices. aux losses are computed redundantly on every core.

Self-contained: shapes hardcoded for B=2, S=2048, H=1024, I=2048, E=8.
"""

import sys

sys.path.insert(0, "/opt/trn_rl_repo")

from contextlib import ExitStack

import numpy as np

import concourse.bacc as bacc
import concourse.mybir as mybir
from concourse import masks, tile
from concourse.bass_types import AP
from concourse.bass_utils import run_bass_kernel_spmd

F32 = mybir.dt.float32
BF16 = mybir.dt.bfloat16
AF = mybir.ActivationFunctionType
OP = mybir.AluOpType

N_CORES = 8
H = 1024
I_DIM = 2048
E = 8
ISL = I_DIM // N_CORES  # shared-expert intermediate slice per core
KH = H // 128           # 8 k-tiles over H
KI = I_DIM // 128       # 16 k-tiles over I
EPS = 1e-6


def _bc_last(ap: AP, n: int) -> AP:
    """Broadcast an AP along a new innermost dim of size n (stride 0)."""
    return AP(ap.tensor, ap.offset, [list(x) for x in ap.ap] + [[0, n]])


def _bc_mid(ap: AP, n: int) -> AP:
    """[128, F] -> [128, n, F] broadcast on the middle dim (stride 0)."""
    a = [list(x) for x in ap.ap]
    return AP(ap.tensor, ap.offset, [a[0], [0, n], *a[1:]])


def build_program(T: int = 4096, tc_tokens: int = 1024, native_silu: bool = True):
    assert T % 1024 == 0 or T in (512,), T
    TC = min(tc_tokens, T)
    NT = T // 128          # token tiles
    NCH = T // TC          # ffn token chunks
    NTS = TC // 128        # token tiles per chunk
    TSL = T // N_CORES     # output slice per core after ReduceScatter

    nc = bacc.Bacc(
        "TRN2",
        target_bir_lowering=False,
        debug=False,
        enable_asserts=True,
        num_devices=N_CORES,
    )

    x_ext = nc.dram_tensor("x", [T, H], F32, kind="ExternalInput").ap()
    rwT_ext = nc.dram_tensor("rwT", [H, E], F32, kind="ExternalInput").ap()
    wg_ext = nc.dram_tensor("wg", [H, I_DIM], F32, kind="ExternalInput").ap()
    wu_ext = nc.dram_tensor("wu", [H, I_DIM], F32, kind="ExternalInput").ap()
    wd_ext = nc.dram_tensor("wd", [I_DIM, H], F32, kind="ExternalInput").ap()
    swg_ext = nc.dram_tensor("swg", [H, ISL], F32, kind="ExternalInput").ap()
    swu_ext = nc.dram_tensor("swu", [H, ISL], F32, kind="ExternalInput").ap()
    swd_ext = nc.dram_tensor("swd", [ISL, H], F32, kind="ExternalInput").ap()
    sel_ext = nc.dram_tensor("sel", [128, E], F32, kind="ExternalInput").ap()
    sgate_ext = nc.dram_tensor("sgate", [128, 1], F32, kind="ExternalInput").ap()

    out_ext = nc.dram_tensor("out_rs", [TSL, H], F32, kind="ExternalOutput").ap()
    aux_ext = nc.dram_tensor("aux", [1, 1], F32, kind="ExternalOutput").ap()

    with tile.TileContext(nc) as tc, ExitStack() as top:
        dram = top.enter_context(tc.tile_pool(name="dram", bufs=1, space="DRAM"))
        RSC = TC // N_CORES                   # rows per core per chunk
        out_accs = [dram.tile([TC, H], F32, name=f"acc{q}", tag=f"acc{q}") for q in range(NCH)]
        rs_outs = [dram.tile([RSC, H], F32, name=f"rsq{q}", tag=f"rsq{q}") for q in range(NCH)]
        wgc = dram.tile([KI, 128, KH * 128], BF16, name="wgc", tag="wgc")
        wuc = dram.tile([KI, 128, KH * 128], BF16, name="wuc", tag="wuc")
        warm_in = dram.tile([N_CORES, 64], F32, name="warm_in", tag="warm_in")
        warm_out = dram.tile([1, 64], F32, name="warm_out", tag="warm_out")

        const_pool = top.enter_context(tc.tile_pool(name="const", bufs=1))
        ident = const_pool.tile([128, 128], F32)
        masks.make_identity(nc, ident[:])
        ones = const_pool.tile([128, 1], F32)
        nc.vector.memset(ones[:], 1.0)
        rwT_sb = const_pool.tile([128, KH, E], F32)
        nc.sync.dma_start(out=rwT_sb[:], in_=rwT_ext.rearrange("(k p) e -> p k e", p=128))
        sel_sb = const_pool.tile([128, E], F32)
        nc.sync.dma_start(out=sel_sb[:], in_=sel_ext)
        sig_sb = const_pool.tile([128, 1], F32)
        sg_in = const_pool.tile([128, 1], F32)
        nc.sync.dma_start(out=sg_in[:], in_=sgate_ext)
        nc.scalar.activation(sig_sb[:], sg_in[:], AF.Sigmoid)

        # Resident activations / weights
        big_pool = top.enter_context(tc.tile_pool(name="resident", bufs=1))
        xT_bf = big_pool.tile([128, KH, T], BF16)          # x^T, bf16
        wd_sb = big_pool.tile([128, KI, H], BF16)          # wd tiles [i_k][i_p, h]
        swg_sb = big_pool.tile([128, KH, ISL], BF16)
        swu_sb = big_pool.tile([128, KH, ISL], BF16)
        swd_sb = big_pool.tile([128, ISL // 128, H], BF16)

        # Router tensors (fp32), persistent until aux is finalized
        rt_pool = top.enter_context(tc.tile_pool(name="router", bufs=1))
        logits_all = rt_pool.tile([128, NT, E], F32)
        probs = rt_pool.tile([128, NT, E], F32)
        mask2 = rt_pool.tile([128, NT, E], F32)
        pl = rt_pool.tile([128, NT, E], F32)
        zsq = rt_pool.tile([128, NT], F32)
        cwc_all = rt_pool.tile([128, NT], F32)

        # ---------------- Phase A: x load + transpose + router logits ----
        with (
            tc.tile_pool(name="xin", bufs=3) as xin_pool,
            tc.tile_pool(name="xtT", bufs=3) as xtT_pool,
            tc.tile_pool(name="trps", bufs=2, space="PSUM") as tr_pool,
            tc.tile_pool(name="lgps", bufs=2, space="PSUM") as lg_pool,
        ):
            for j in range(NT):
                xt = xin_pool.tile([128, H], F32)
                nc.sync.dma_start(out=xt[:], in_=x_ext[j * 128:(j + 1) * 128, :])
                tr_ps = tr_pool.tile([128, H], F32)
                for k in range(KH):
                    nc.tensor.transpose(
                        tr_ps[:, k * 128:(k + 1) * 128],
                        xt[:, k * 128:(k + 1) * 128],
                        ident[:],
                    )
                xtT = xtT_pool.tile([128, H], F32)
                nc.scalar.activation(xtT[:], tr_ps[:], AF.Copy)
                nc.vector.tensor_copy(
                    xT_bf[:, :, j * 128:(j + 1) * 128],
                    xtT[:].rearrange("p (k t) -> p k t", k=KH),
                )
                lg_ps = lg_pool.tile([128, E], F32)
                for k in range(KH):
                    nc.tensor.matmul(
                        lg_ps[:],
                        xtT[:, k * 128:(k + 1) * 128],
                        rwT_sb[:, k, :],
                        start=(k == 0),
                        stop=(k == KH - 1),
                    )
                nc.vector.tensor_scalar(
                    out=logits_all[:, j, :], in0=lg_ps[:],
                    scalar1=50.0, scalar2=-50.0, op0=OP.min, op1=OP.max,
                )

        wz = const_pool.tile([N_CORES, 64], F32)
        nc.vector.memset(wz[:], 0.0)
        nc.sync.dma_start(out=warm_in[:], in_=wz[:])
        nc.gpsimd.collective_compute(
            "ReduceScatter",
            OP.add,
            ins=[warm_in.opt()],
            outs=[warm_out.opt()],
            replica_groups=[list(range(N_CORES))],
        )

        # ---------------- Router math (batched over all tokens) ----------
        with tc.tile_pool(name="rtmp", bufs=1) as rt:
            mx = rt.tile([128, NT], F32)
            nc.vector.tensor_reduce(mx[:], logits_all[:], axis=mybir.AxisListType.X, op=OP.max)
            shifted = rt.tile([128, NT, E], F32)
            nc.vector.tensor_tensor(shifted[:], logits_all[:], _bc_last(mx[:], E), OP.subtract)
            exps = rt.tile([128, NT, E], F32)
            nc.scalar.activation(exps[:], shifted[:], AF.Exp)
            sums = rt.tile([128, NT], F32)
            nc.vector.tensor_reduce(sums[:], exps[:], axis=mybir.AxisListType.X, op=OP.add)
            rcp = rt.tile([128, NT], F32)
            nc.vector.reciprocal(rcp[:], sums[:])
            nc.vector.tensor_tensor(probs[:], exps[:], _bc_last(rcp[:], E), OP.mult)
            # z-loss: lse = ln(sum) + max, squared
            lns = rt.tile([128, NT], F32)
            nc.scalar.activation(lns[:], sums[:], AF.Ln)
            lse = rt.tile([128, NT], F32)
            nc.vector.tensor_tensor(lse[:], lns[:], mx[:], OP.add)
            nc.vector.tensor_tensor(zsq[:], lse[:], lse[:], OP.mult)
            # top-2 selection on LOGITS (same order as probs; avoids any
            # dependence of the selection on exp-LUT rounding)
            m1 = rt.tile([128, NT], F32)
            nc.vector.tensor_reduce(m1[:], logits_all[:], axis=mybir.AxisListType.X, op=OP.max)
            eq1 = rt.tile([128, NT, E], F32)
            nc.vector.tensor_tensor(eq1[:], logits_all[:], _bc_last(m1[:], E), OP.is_ge)
            pm = rt.tile([128, NT, E], F32)
            nc.vector.tensor_scalar_mul(pm[:], eq1[:], 1.0e9)
            nc.vector.tensor_tensor(pm[:], logits_all[:], pm[:], OP.subtract)
            m2 = rt.tile([128, NT], F32)
            nc.vector.tensor_reduce(m2[:], pm[:], axis=mybir.AxisListType.X, op=OP.max)
            nc.vector.tensor_tensor(mask2[:], logits_all[:], _bc_last(m2[:], E), OP.is_ge)
            # cw = probs * mask2 / (sum of selected probs + EPS)
            cw = rt.tile([128, NT, E], F32)
            nc.vector.tensor_tensor(cw[:], probs[:], mask2[:], OP.mult)
            den = rt.tile([128, NT], F32)
            nc.vector.tensor_reduce(den[:], cw[:], axis=mybir.AxisListType.X, op=OP.add)
            nc.vector.tensor_scalar_add(den[:], den[:], EPS)
            rd = rt.tile([128, NT], F32)
            nc.vector.reciprocal(rd[:], den[:])
            nc.vector.tensor_tensor(cw[:], cw[:], _bc_last(rd[:], E), OP.mult)
            cwsel = rt.tile([128, NT, E], F32)
            nc.vector.tensor_tensor(cwsel[:], cw[:], _bc_mid(sel_sb[:], NT), OP.mult)
            nc.vector.tensor_reduce(cwc_all[:], cwsel[:], axis=mybir.AxisListType.X, op=OP.add)
            # entropy terms
            psafe = rt.tile([128, NT, E], F32)
            nc.vector.tensor_scalar_max(psafe[:], probs[:], EPS)
            lp = rt.tile([128, NT, E], F32)
            nc.scalar.activation(lp[:], psafe[:], AF.Ln)
            nc.vector.tensor_tensor(pl[:], psafe[:], lp[:], OP.mult)

        # ---------------- aux loss reductions ----------------------------
        with (
            tc.tile_pool(name="stps", bufs=1, space="PSUM") as st_pool,
            tc.tile_pool(name="stsb", bufs=1) as st_sb_pool,
        ):
            stm = st_pool.tile([1, NT * E], F32)
            stp = st_pool.tile([1, NT * E], F32)
            stz = st_pool.tile([1, NT], F32)
            stl = st_pool.tile([1, NT * E], F32)
            nc.tensor.matmul(stm[:], ones[:], mask2[:].rearrange("p a b -> p (a b)"), start=True, stop=True)
            nc.tensor.matmul(stp[:], ones[:], probs[:].rearrange("p a b -> p (a b)"), start=True, stop=True)
            nc.tensor.matmul(stz[:], ones[:], zsq[:], start=True, stop=True)
            nc.tensor.matmul(stl[:], ones[:], pl[:].rearrange("p a b -> p (a b)"), start=True, stop=True)

            smv = st_sb_pool.tile([1, NT * E], F32)
            spv = st_sb_pool.tile([1, NT * E], F32)
            szv = st_sb_pool.tile([1, NT], F32)
            slv = st_sb_pool.tile([1, NT * E], F32)
            nc.scalar.activation(smv[:], stm[:], AF.Copy)
            nc.scalar.activation(spv[:], stp[:], AF.Copy)
            nc.scalar.activation(szv[:], stz[:], AF.Copy)
            nc.scalar.activation(slv[:], stl[:], AF.Copy)

            def _sum_over_tiles(dst, src):
                # src [1, NT*E] viewed as [1, E, NT] (strided) -> reduce X
                v = AP(src.tensor, src.offset, [list(src.ap[0]), [1, E], [E, NT]])
                nc.vector.tensor_reduce(dst, v, axis=mybir.AxisListType.X, op=OP.add)

            tpe = st_sb_pool.tile([1, E], F32)
            avg = st_sb_pool.tile([1, E], F32)
            _sum_over_tiles(tpe[:], smv[:])
            _sum_over_tiles(avg[:], spv[:])
            nc.vector.tensor_scalar_mul(tpe[:], tpe[:], 1.0 / (2 * T))
            nc.vector.tensor_scalar_mul(avg[:], avg[:], 1.0 / T)
            prod = st_sb_pool.tile([1, E], F32)
            nc.vector.tensor_tensor(prod[:], tpe[:], avg[:], OP.mult)
            lb = st_sb_pool.tile([1, 1], F32)
            nc.vector.tensor_reduce(lb[:], prod[:], axis=mybir.AxisListType.X, op=OP.add)
            nc.vector.tensor_scalar_mul(lb[:], lb[:], float(E))

            zt = st_sb_pool.tile([1, 1], F32)
            nc.vector.tensor_reduce(zt[:], szv[:], axis=mybir.AxisListType.X, op=OP.add)
            nc.vector.tensor_scalar_mul(zt[:], zt[:], 0.001 / T)

            el = st_sb_pool.tile([1, 1], F32)
            nc.vector.tensor_reduce(el[:], slv[:], axis=mybir.AxisListType.X, op=OP.add)
            # entropy = -sum/T ; el = (ln E - entropy) * 0.01 = sum*(0.01/T) + 0.01*lnE
            nc.vector.tensor_scalar(
                out=el[:], in0=el[:], scalar1=0.01 / T, scalar2=float(0.01 * np.log(E)),
                op0=OP.mult, op1=OP.add,
            )

            ug = st_sb_pool.tile([1, E], F32)
            nc.vector.tensor_scalar(out=ug[:], in0=tpe[:], scalar1=0.01, scalar2=None, op0=OP.is_gt)
            ul = st_sb_pool.tile([1, 1], F32)
            nc.vector.tensor_reduce(ul[:], ug[:], axis=mybir.AxisListType.X, op=OP.add)
            # util = (1 - usage/E) * 0.1 = -us*(0.1/E) + 0.1
            nc.vector.tensor_scalar(
                out=ul[:], in0=ul[:], scalar1=-0.1 / E, scalar2=0.1, op0=OP.mult, op1=OP.add,
            )

            auxv = st_sb_pool.tile([1, 1], F32)
            nc.vector.tensor_tensor(auxv[:], lb[:], zt[:], OP.add)
            nc.vector.tensor_tensor(auxv[:], auxv[:], el[:], OP.add)
            nc.vector.tensor_tensor(auxv[:], auxv[:], ul[:], OP.add)
            nc.vector.tensor_scalar(
                out=auxv[:], in0=auxv[:], scalar1=100.0, scalar2=0.0, op0=OP.min, op1=OP.max,
            )
            nc.sync.dma_start(out=aux_ext, in_=auxv[:])

        nc.gpsimd.dma_start(out=wd_sb[:], in_=wd_ext.rearrange("(k p) h -> p k h", p=128))
        nc.gpsimd.dma_start(out=swg_sb[:], in_=swg_ext.rearrange("(k p) i -> p k i", p=128))
        nc.gpsimd.dma_start(out=swu_sb[:], in_=swu_ext.rearrange("(k p) i -> p k i", p=128))
        nc.gpsimd.dma_start(out=swd_sb[:], in_=swd_ext.rearrange("(k p) h -> p k h", p=128))

        # ---------------- Phase B: routed expert FFN (dense, masked) -----
        # ---------------- Phase C: shared expert slice --------------------
        with (
            tc.tile_pool(name="wstream", bufs=3) as wstream,
            tc.tile_pool(name="gact", bufs=3) as gact_pool,
            tc.tile_pool(name="hT", bufs=1) as hT_pool,
            tc.tile_pool(name="osb", bufs=4) as osb_pool,
            tc.tile_pool(name="gps", bufs=1, space="PSUM") as g_pool,
            tc.tile_pool(name="ups", bufs=1, space="PSUM") as u_pool,
            tc.tile_pool(name="ops", bufs=2, space="PSUM") as o_pool,
        ):
            hT = hT_pool.tile([128, KI, TC], BF16, tag="hT")
            hsT = hT_pool.tile([128, ISL // 128, TC], BF16, tag="hsT")

            def _silu_mul(dst, g_ps, u_ps):
                ga = gact_pool.tile([128, TC], BF16, tag="ga")
                if native_silu:
                    nc.scalar.activation(ga[:], g_ps[:], AF.Silu)
                else:
                    sgm = gact_pool.tile([128, TC], F32, tag="sgm")
                    nc.scalar.activation(sgm[:], g_ps[:], AF.Sigmoid)
                    nc.vector.tensor_tensor(ga[:], sgm[:], g_ps[:], OP.mult)
                nc.vector.tensor_tensor(dst, ga[:], u_ps[:], OP.mult)
            for ch in range(NCH):
                t0 = ch * TC
                # gate/up for the routed expert
                for i in range(KI):
                    wg_sb = wstream.tile([128, KH, 128], BF16, tag="wg")
                    wu_sb = wstream.tile([128, KH, 128], BF16, tag="wu")
                    if ch == 0:
                        # first pass: cast-load fp32 -> bf16 (SWDGE), then cache
                        # the bf16 tiles in DRAM so later chunks reload via
                        # HWDGE and keep the gpsimd queue free for collectives
                        nc.gpsimd.dma_start(
                            out=wg_sb[:],
                            in_=AP(wg_ext.tensor, i * 128, [[I_DIM, 128], [128 * I_DIM, KH], [1, 128]]),
                        )
                        nc.gpsimd.dma_start(
                            out=wu_sb[:],
                            in_=AP(wu_ext.tensor, i * 128, [[I_DIM, 128], [128 * I_DIM, KH], [1, 128]]),
                        )
                        if NCH > 1:
                            nc.scalar.dma_start(out=wgc[i], in_=wg_sb[:])
                            nc.scalar.dma_start(out=wuc[i], in_=wu_sb[:])
                    else:
                        nc.sync.dma_start(out=wg_sb[:].rearrange("p a b -> p (a b)"), in_=wgc[i])
                        nc.sync.dma_start(out=wu_sb[:].rearrange("p a b -> p (a b)"), in_=wuc[i])
                    g_ps = g_pool.tile([128, TC], F32, tag="g")
                    u_ps = u_pool.tile([128, TC], F32, tag="u")
                    for half in range(TC // 512):
                        hs = slice(half * 512, (half + 1) * 512)
                        for k in range(KH):
                            nc.tensor.matmul(
                                g_ps[:, hs], wg_sb[:, k, :],
                                xT_bf[:, k, t0 + half * 512: t0 + (half + 1) * 512],
                                start=(k == 0), stop=(k == KH - 1),
                            )
                        for k in range(KH):
                            nc.tensor.matmul(
                                u_ps[:, hs], wu_sb[:, k, :],
                                xT_bf[:, k, t0 + half * 512: t0 + (half + 1) * 512],
                                start=(k == 0), stop=(k == KH - 1),
                            )
                    _silu_mul(hT[:, i, :], g_ps, u_ps)
                # gate/up for the shared expert slice
                for i2 in range(ISL // 128):
                    g_ps = g_pool.tile([128, TC], F32, tag="g")
                    u_ps = u_pool.tile([128, TC], F32, tag="u")
                    for half in range(TC // 512):
                        hs = slice(half * 512, (half + 1) * 512)
                        for k in range(KH):
                            nc.tensor.matmul(
                                g_ps[:, hs], swg_sb[:, k, i2 * 128:(i2 + 1) * 128],
                                xT_bf[:, k, t0 + half * 512: t0 + (half + 1) * 512],
                                start=(k == 0), stop=(k == KH - 1),
                            )
                        for k in range(KH):
                            nc.tensor.matmul(
                                u_ps[:, hs], swu_sb[:, k, i2 * 128:(i2 + 1) * 128],
                                xT_bf[:, k, t0 + half * 512: t0 + (half + 1) * 512],
                                start=(k == 0), stop=(k == KH - 1),
                            )
                    _silu_mul(hsT[:, i2, :], g_ps, u_ps)
                # down projections
                for ts_ in range(NTS):
                    jj = ch * NTS + ts_
                    tsl = slice(ts_ * 128, (ts_ + 1) * 128)
                    o_ps = o_pool.tile([128, H], F32, tag="o")
                    for half in range(H // 512):
                        hs = slice(half * 512, (half + 1) * 512)
                        for i in range(KI):
                            nc.tensor.matmul(
                                o_ps[:, hs], hT[:, i, tsl], wd_sb[:, i, hs],
                                start=(i == 0), stop=(i == KI - 1),
                            )
                    o_sb = osb_pool.tile([128, H], F32, tag="osb")
                    nc.vector.tensor_scalar_mul(o_sb[:], o_ps[:], cwc_all[:, jj:jj + 1])
                    nc.scalar.dma_start(out=out_accs[ch][ts_ * 128:(ts_ + 1) * 128, :], in_=o_sb[:])
                    # shared down, accumulated into out_acc by DMA
                    o2_ps = o_pool.tile([128, H], F32, tag="o")
                    for half in range(H // 512):
                        hs = slice(half * 512, (half + 1) * 512)
                        for i2 in range(ISL // 128):
                            nc.tensor.matmul(
                                o2_ps[:, hs], hsT[:, i2, tsl], swd_sb[:, i2, hs],
                                start=(i2 == 0), stop=(i2 == ISL // 128 - 1),
                            )
                    o2_sb = osb_pool.tile([128, H], F32, tag="osb")
                    nc.vector.tensor_scalar_mul(o2_sb[:], o2_ps[:], sig_sb[:])
                    nc.gpsimd.dma_start(
                        out=out_accs[ch][ts_ * 128:(ts_ + 1) * 128, :], in_=o2_sb[:],
                        accum_op=OP.add,
                    )
                # this chunk of tokens is complete: reduce-scatter it now so the
                # collective overlaps the next chunk's compute
                nc.gpsimd.collective_compute(
                    "ReduceScatter",
                    OP.add,
                    ins=[out_accs[ch].opt()],
                    outs=[rs_outs[ch].opt()],
                    replica_groups=[list(range(N_CORES))],
                )
                nc.scalar.dma_start(
                    out=out_ext[ch * RSC:(ch + 1) * RSC, :], in_=rs_outs[ch][:]
                )


    nc.compile()
    return nc


_CACHED = {}


def _get_program(T):
    if T not in _CACHED:
        _CACHED[T] = build_program(T)
    return _CACHED[T]


def make_in_maps(inputs: dict, T: int):
    x = np.ascontiguousarray(np.asarray(inputs["hidden_states"], dtype=np.float32).reshape(T, H))
    rwT = np.ascontiguousarray(np.asarray(inputs["router_w"], dtype=np.float32).T)
    wg = np.asarray(inputs["wg"], dtype=np.float32)
    wu = np.asarray(inputs["wu"], dtype=np.float32)
    wd = np.asarray(inputs["wd"], dtype=np.float32)
    swg = np.asarray(inputs["shared_wg"], dtype=np.float32)
    swu = np.asarray(inputs["shared_wu"], dtype=np.float32)
    swd = np.asarray(inputs["shared_wd"], dtype=np.float32)
    gate = float(np.asarray(inputs["shared_gate"]).reshape(-1)[0])
    in_maps = []
    for c in range(N_CORES):
        sel = np.zeros((128, E), dtype=np.float32)
        sel[:, c] = 1.0
        in_maps.append({
            "x": x,
            "rwT": rwT,
            "wg": np.ascontiguousarray(wg[c]),
            "wu": np.ascontiguousarray(wu[c]),
            "wd": np.ascontiguousarray(wd[c]),
            "swg": np.ascontiguousarray(swg[:, c * ISL:(c + 1) * ISL]),
            "swu": np.ascontiguousarray(swu[:, c * ISL:(c + 1) * ISL]),
            "swd": np.ascontiguousarray(swd[c * ISL:(c + 1) * ISL, :]),
            "sel": sel,
            "sgate": np.full((128, 1), gate, dtype=np.float32),
        })
    return in_maps


def assemble_out(results, T):
    # core c's "out_rs" rows are [chunk, 128] with global token
    # t = chunk*TC + c*128 + r (chunked ReduceScatter layout)
    TC = min(1024, T)
    NCH = T // TC
    stack = np.stack([results[c]["out_rs"] for c in range(N_CORES)], axis=0)
    stack = stack.reshape(N_CORES, NCH, TC // N_CORES, H).transpose(1, 0, 2, 3)
    return stack.reshape(T, H)


def kernel(**inputs):
    hs = np.asarray(inputs["hidden_states"])
    B, S, _ = hs.shape
    T = B * S
    nc = _get_program(T)
    in_maps = make_in_maps(inputs, T)
    res = run_bass_kernel_spmd(nc, in_maps, list(range(N_CORES)))
    out = assemble_out(res.results, T)
    aux = np.float32(res.results[0]["aux"].reshape(())[()])
    return out.reshape(B, S, H), aux
